# revision 32
# baseline (speedup 1.0000x reference)
"""BERT-base + CRF loss kernel for 8 Trainium2 NeuronCores.

Strategy (data-parallel over batch, B=8 -> 1 sequence per core):
  - Host: embedding gather (+pos/+type), pack weights into bf16 DMA blobs.
  - Device (per core): 12-layer BERT encoder + classifier head on one
    [512, 768] sequence, activations kept in transposed [H, T] layout
    (hidden on partitions) so no transposes are ever needed. Attention
    computes exp(S^T) unnormalized; a ones-column baked into V makes the
    softmax denominator fall out of the AV matmul; normalization is fused
    into the ctx eviction. Matmuls in bf16 (1 col/cycle), fp32 PSUM.
  - Host: CRF negative log-likelihood on the [B, 512, 9] emissions
    (tiny sequential scan, bad fit for the PE) and the final scalar sum.
"""

import os
import sys

for _p in ("/opt/trn_rl_repo", "/root/.axon_site/_ro/trn_rl_repo"):
    if os.path.isdir(_p) and _p not in sys.path:
        sys.path.insert(0, _p)

import numpy as np
import ml_dtypes

import concourse.bass as bass
import concourse.tile as tile
from concourse import bacc, mybir

BF16 = mybir.dt.bfloat16
F32 = mybir.dt.float32
AF = mybir.ActivationFunctionType
OP = mybir.AluOpType

V, H, NL, NH, DH, FF, MAXPOS, NT = 30522, 768, 12, 12, 64, 3072, 512, 9
B, S = 8, 512
EPS = 1e-12
KT = H // 128        # 6 hidden k-tiles
FKT = FF // 128      # 24 ffn k-tiles
TT = S // 128        # 4 token tiles
WAC = 18432          # blob cols per layer
HALF = WAC // 2      # 9216
SPL = 72             # smallp cols per layer
SPC = 12 + SPL * NL  # smallp total cols


def _build_program(flags):
    """flags = (zq, zk, zv, zo, zi, zo2, ln_triv); z* True means that bias is
    all-zero (skip adding it), ln_triv True means every LN gamma==1, beta==0."""
    zq, zk, zv, zo, zi, zo2, ln_triv = flags
    nc = bacc.Bacc("TRN2", target_bir_lowering=False, debug=False)

    x0T = nc.dram_tensor("x0T", [H, S], BF16, kind="ExternalInput")
    wA = nc.dram_tensor("wA", [NL, 128, WAC], BF16, kind="ExternalInput")
    wB = nc.dram_tensor("wB", [NL, 128, WAC], BF16, kind="ExternalInput")
    wC = nc.dram_tensor("wC", [NL, 128, WAC], BF16, kind="ExternalInput")
    need_smallp = (not ln_triv) or not (zq and zk and zo and zi and zo2)
    smallp = (
        nc.dram_tensor("smallp", [128, SPC], F32, kind="ExternalInput")
        if need_smallp
        else None
    )
    smallb = nc.dram_tensor("smallb", [128, KT * NT], BF16, kind="ExternalInput")
    smallrow = (
        nc.dram_tensor("smallrow", [1, NL * H], F32, kind="ExternalInput")
        if not zv
        else None
    )
    logits = nc.dram_tensor("logits", [S, NT], F32, kind="ExternalOutput")

    with tile.TileContext(nc) as tc:
        with (
            tc.tile_pool(name="sb", bufs=1) as sb,
            tc.tile_pool(name="ps", bufs=1, space="PSUM") as ps,
        ):
            ones1 = sb.tile([128, 1], BF16, name="ones1")
            nc.vector.memset(ones1, 1.0)
            onesr = sb.tile([1, 128], F32, name="onesr")
            nc.vector.memset(onesr, 1.0)
            eps_sc = sb.tile([1, 1], F32, name="eps_sc")
            nc.vector.memset(eps_sc, EPS)
            warm_sc = sb.tile([1, 1], F32, name="warm_sc")
            nc.vector.memset(warm_sc, 1.0)

            def act_warmup(i, func):
                # touch an ACT LUT while the PE is busy elsewhere so the
                # 1.3us table load doesn't land on a critical path later
                w1 = sb.tile([1, 1], F32, name=f"aw{i}", tag="warm", bufs=2)
                nc.scalar.activation(w1, warm_sc, func)
            sp_sb = None
            if smallp is not None:
                sp_sb = sb.tile([128, SPC], F32, name="sp_sb")
                nc.sync.dma_start(sp_sb, smallp[:, :])
            sb_cls = sb.tile([128, KT * NT], BF16, name="sb_cls")
            nc.sync.dma_start(sb_cls, smallb[:, :])
            srow_sb = None
            if smallrow is not None:
                srow_sb = sb.tile([1, NL * H], F32, name="srow_sb")
                nc.sync.dma_start(srow_sb, smallrow[:, :])

            def pcol(off):
                assert sp_sb is not None
                return sp_sb[:, off : off + 1]

            row_i = [0]

            def rowtile(nm):
                row_i[0] += 1
                return sb.tile([1, S], F32, name=f"r{nm}{row_i[0]}", tag="row", bufs=6)

            ln_i = [0]

            def layer_norm(r_tiles, goff, boff):
                """r_tiles: 6 bf16 [128, S] tiles. Returns 6 new bf16 tiles.

                Critical-path layout: the -mean broadcast happens as soon as
                the plain sum is done, so the centering pass (pass 1) for
                every tile overlaps the sum-of-squares / rstd chain; only the
                final multiply by rstd (pass 2) waits for the full chain.
                Broadcasts are ones-matmuls on the PE (keeps HAM warm during
                the LN bubble)."""
                ln_i[0] += 1
                i = ln_i[0]
                s_ps = ps.tile([1, S], F32, name=f"lns{i}", tag="psrow", bufs=1)
                s2_ps = ps.tile([1, S], F32, name=f"lns2{i}", tag="pscore", bufs=3)
                for k in range(KT):
                    nc.tensor.matmul(
                        s_ps, ones1, r_tiles[k], start=(k == 0), stop=(k == KT - 1)
                    )
                # -mean row + its broadcast (early; feeds pass 1). The
                # broadcast is a K=1 fp32 ones-matmul into PSUM: exact, and it
                # is real PE activity in the middle of the LN bubble.
                mb_row = rowtile("mb")
                nc.vector.tensor_scalar_mul(mb_row, s_ps, -1.0 / H)
                mb_b = sb.tile([128, S], F32, name=f"lnmb{i}", tag="bc", bufs=4)
                nc.gpsimd.partition_broadcast(mb_b, mb_row)
                # sum of squares -> rstd
                for k in range(KT):
                    sq = sb.tile([128, S], BF16, name=f"lnsq{i}_{k}", tag="xsq", bufs=2)
                    nc.scalar.activation(sq, r_tiles[k], AF.Square)
                    nc.tensor.matmul(
                        s2_ps, ones1, sq, start=(k == 0), stop=(k == KT - 1)
                    )
                mm_row = rowtile("mm")  # mb^2 = (s/H)^2
                nc.vector.tensor_mul(mm_row, mb_row, mb_row)
                u_row = rowtile("u")  # u = s2 - H*mb^2  (var = u/H)
                nc.vector.scalar_tensor_tensor(
                    u_row, mm_row, -float(H), s2_ps, op0=OP.mult, op1=OP.add
                )
                std_row = rowtile("std")  # sqrt(u/H + eps)
                nc.scalar.activation(
                    std_row, u_row, AF.Sqrt, bias=eps_sc[:, :], scale=1.0 / H
                )
                rr_row = rowtile("rr")
                nc.vector.reciprocal_approx_fast(rr_row, std_row)
                rb_b = sb.tile([128, S], F32, name=f"lnrb{i}", tag="bc", bufs=4)
                nc.gpsimd.partition_broadcast(rb_b, rr_row)
                # dummy weight loads: PE activity pulses spaced through the LN
                # bubble (dep on successive row tiles of the rstd chain) so
                # the HAM monitor never demotes the clock. Values are unused —
                # every matmul issues its own LDWEIGHTS.
                for src in (u_row, std_row, rr_row):
                    v = src.bitcast(BF16)
                    for b in range(3):
                        nc.tensor.ldweights(v[:1, b * 128 : (b + 1) * 128])
                out = []
                for k in range(KT):
                    t1 = sb.tile([128, S], F32, name=f"lnt{i}_{k}", tag="lnt", bufs=6)
                    nc.vector.tensor_add(t1, r_tiles[k], mb_b)  # pass 1 (early)
                    o = sb.tile([128, S], BF16, name=f"lno{i}_{k}", tag="h", bufs=16)
                    if ln_triv:
                        nc.vector.tensor_mul(o, t1, rb_b)  # pass 2
                    else:
                        t2 = sb.tile(
                            [128, S], F32, name=f"lnu{i}_{k}", tag="lnt2", bufs=2
                        )
                        nc.vector.tensor_mul(t2, t1, rb_b)
                        nc.vector.tensor_scalar(
                            o, t2, pcol(goff + k), pcol(boff + k),
                            op0=OP.mult, op1=OP.add,
                        )
                    out.append(o)
                return out

            # ---- embeddings LN ----
            x0 = []
            for k in range(KT):
                xt = sb.tile([128, S], BF16, name=f"x0_{k}", tag="h", bufs=16)
                nc.sync.dma_start(xt, x0T[k * 128 : (k + 1) * 128, :])
                x0.append(xt)
            h = layer_norm(x0, 0, 6)
            act_warmup(99, AF.Exp)

            for l in range(NL):
                base = 12 + SPL * l
                wa1 = sb.tile([128, HALF], BF16, name=f"wa1_{l}", tag="w", bufs=4)
                nc.sync.dma_start(wa1, wA[l, :, 0:HALF])
                wa2 = sb.tile([128, HALF], BF16, name=f"wa2_{l}", tag="w", bufs=4)
                nc.sync.dma_start(wa2, wA[l, :, HALF:WAC])

                # Q^T, K^T: [H, S] tiles
                qt, kt = [], []
                for which, dst, woff, bias_off, bz in (
                    (0, qt, 0, base + 24, zq),
                    (1, kt, 4608, base + 30, zk),
                ):
                    for m in range(KT):
                        pq = ps.tile(
                            [128, S], F32, name=f"pq{l}_{which}_{m}", tag="pp", bufs=2
                        )
                        for k in range(KT):
                            nc.tensor.matmul(
                                pq,
                                wa1[:, woff + k * H + m * 128 : woff + k * H + (m + 1) * 128],
                                h[k],
                                start=(k == 0),
                                stop=(k == KT - 1),
                            )
                        o = sb.tile(
                            [128, S], BF16, name=f"qk{l}_{which}_{m}", tag="qkt",
                            bufs=14,
                        )
                        if bz:
                            nc.scalar.activation(o, pq, AF.Copy)
                        else:
                            nc.scalar.activation(
                                o, pq, AF.Identity, bias=pcol(bias_off + m)
                            )
                        dst.append(o)

                # V in token-major layout with a ones column per head:
                # v_sb[t] is [128 tokens, 12 heads, 65] (col 64 == 1.0)
                v_sb = []
                for t in range(TT):
                    vt = sb.tile([128, NH, DH + 1], BF16, name=f"v{l}_{t}", tag="v",
                                 bufs=5)
                    nc.vector.memset(vt[:, :, DH : DH + 1], 1.0)
                    v_sb.append(vt)
                for t in range(TT):
                    for n in range(2):
                        pv = ps.tile(
                            [128, 384], F32, name=f"pv{l}_{t}_{n}", tag="pp", bufs=2
                        )
                        for k in range(KT):
                            nc.tensor.matmul(
                                pv,
                                h[k][:, t * 128 : (t + 1) * 128],
                                wa2[:, k * H + n * 384 : k * H + (n + 1) * 384],
                                start=(k == 0),
                                stop=(k == KT - 1),
                            )
                        dstv = v_sb[t][:, n * 6 : (n + 1) * 6, 0:DH]
                        pv3 = pv.rearrange("p (a b) -> p a b", a=6)
                        if zv:
                            nc.scalar.activation(dstv, pv3, AF.Copy)
                        else:
                            assert srow_sb is not None
                            bvb = sb.tile(
                                [128, 384], F32, name=f"bvb{l}_{t}_{n}", tag="bvb",
                                bufs=2,
                            )
                            nc.gpsimd.partition_broadcast(
                                bvb, srow_sb[:, l * H + n * 384 : l * H + (n + 1) * 384]
                            )
                            nc.vector.tensor_add(
                                dstv, pv3, bvb.rearrange("p (a b) -> p a b", a=6)
                            )

                # attention, head by head; output ctx^T [H, S]
                ctxt = [
                    sb.tile([128, S], BF16, name=f"ctx{l}_{k}", tag="ctx", bufs=8)
                    for k in range(KT)
                ]
                # heads in groups of 3: emit every score matmul of the group
                # before any AV matmul, so the in-order PE stream has dense
                # work while the ACT engine chases the exp evictions
                for hg in range(0, NH, 3):
                    group = list(range(hg, hg + 3))
                    e_all = {}
                    for hd in group:
                        kti = hd // 2
                        poff = (hd % 2) * DH
                        kt_sl = kt[kti][poff : poff + DH, :]
                        qt_sl = qt[kti][poff : poff + DH, :]
                        e_tiles = []
                        for t4 in range(TT):
                            pss = ps.tile(
                                [128, S], F32, name=f"psS{l}_{hd}_{t4}", tag="pscore",
                                bufs=3,
                            )
                            nc.tensor.matmul(
                                pss,
                                kt_sl[:, t4 * 128 : (t4 + 1) * 128],
                                qt_sl,
                                start=True,
                                stop=True,
                            )
                            et = sb.tile(
                                [128, S], BF16, name=f"e{l}_{hd}_{t4}", tag="e",
                                bufs=16,
                            )
                            nc.scalar.activation(et, pss, AF.Exp, scale=0.125)
                            e_tiles.append(et)
                        e_all[hd] = e_tiles
                    for hd in group:
                        kti = hd // 2
                        poff = (hd % 2) * DH
                        pc = ps.tile([DH + 1, S], F32, name=f"pc{l}_{hd}", tag="pctx",
                                     bufs=2)
                        for t4 in range(TT):
                            nc.tensor.matmul(
                                pc,
                                v_sb[t4][:, hd, :],
                                e_all[hd][t4],
                                start=(t4 == 0),
                                stop=(t4 == TT - 1),
                            )
                        # free the PSUM bank fast: evict unnormalized ctx via
                        # ACT, then normalize off the PE/PSUM critical path on
                        # DVE. (reciprocal_approx_fast drops nonzero partition
                        # offsets, so stage Z to a partition-0 tile first — on
                        # ACT, keeping the per-head tail off the DVE queue)
                        zz = rowtile("zz")
                        nc.scalar.activation(zz, pc[DH : DH + 1, :], AF.Copy)
                        rz = rowtile("rz")
                        nc.vector.reciprocal_approx_fast(rz, zz)
                        cu = sb.tile([DH, S], F32, name=f"cu{l}_{hd}", tag="cu",
                                     bufs=4)
                        nc.scalar.activation(cu, pc[0:DH, :], AF.Copy)
                        rzb = sb.tile([DH, S], F32, name=f"rzb{l}_{hd}", tag="rzb",
                                      bufs=3)
                        nc.gpsimd.partition_broadcast(rzb, rz)
                        nc.vector.tensor_mul(
                            ctxt[kti][poff : poff + DH, :], cu, rzb
                        )

                # Wo projection + residual
                r1 = []
                for m in range(KT):
                    po = ps.tile([128, S], F32, name=f"po{l}_{m}", tag="pp", bufs=2)
                    for k in range(KT):
                        nc.tensor.matmul(
                            po,
                            wa2[:, 4608 + k * H + m * 128 : 4608 + k * H + (m + 1) * 128],
                            ctxt[k],
                            start=(k == 0),
                            stop=(k == KT - 1),
                        )
                    o = sb.tile([128, S], BF16, name=f"r1_{l}_{m}", tag="h", bufs=16)
                    if zo:
                        nc.vector.tensor_add(o, h[m], po)
                    else:
                        nc.vector.scalar_tensor_tensor(
                            o, po, pcol(base + 36 + m), h[m], op0=OP.add, op1=OP.add
                        )
                    r1.append(o)
                h1 = layer_norm(r1, base + 0, base + 6)

                # FFN
                wb1 = sb.tile([128, HALF], BF16, name=f"wb1_{l}", tag="w", bufs=4)
                nc.sync.dma_start(wb1, wB[l, :, 0:HALF])
                wb2 = sb.tile([128, HALF], BF16, name=f"wb2_{l}", tag="w", bufs=4)
                nc.sync.dma_start(wb2, wB[l, :, HALF:WAC])
                g = []
                for fm in range(FKT):
                    pg = ps.tile([128, S], F32, name=f"pg{l}_{fm}", tag="pp", bufs=2)
                    for k in range(KT):
                        wh = wb1 if k < 3 else wb2
                        off = (k % 3) * FF + fm * 128
                        nc.tensor.matmul(
                            pg, wh[:, off : off + 128], h1[k],
                            start=(k == 0), stop=(k == KT - 1),
                        )
                    go = sb.tile([128, S], BF16, name=f"g{l}_{fm}", tag="g", bufs=24)
                    if zi:
                        nc.scalar.activation(go, pg, AF.Gelu)
                    else:
                        nc.scalar.activation(go, pg, AF.Gelu, bias=pcol(base + 48 + fm))
                    g.append(go)

                wc1 = sb.tile([128, HALF], BF16, name=f"wc1_{l}", tag="w", bufs=4)
                nc.sync.dma_start(wc1, wC[l, :, 0:HALF])
                wc2 = sb.tile([128, HALF], BF16, name=f"wc2_{l}", tag="w", bufs=4)
                nc.sync.dma_start(wc2, wC[l, :, HALF:WAC])
                r2 = []
                for m in range(KT):
                    pf = ps.tile([128, S], F32, name=f"pf{l}_{m}", tag="pp", bufs=2)
                    for k in range(FKT):
                        wh = wc1 if k < 12 else wc2
                        off = (k % 12) * H + m * 128
                        nc.tensor.matmul(
                            pf, wh[:, off : off + 128], g[k],
                            start=(k == 0), stop=(k == FKT - 1),
                        )
                    o = sb.tile([128, S], BF16, name=f"r2_{l}_{m}", tag="h", bufs=16)
                    if zo2:
                        nc.vector.tensor_add(o, h1[m], pf)
                    else:
                        nc.vector.scalar_tensor_tensor(
                            o, pf, pcol(base + 42 + m), h1[m], op0=OP.add, op1=OP.add
                        )
                    r2.append(o)
                h = layer_norm(r2, base + 12, base + 18)
                if l + 1 < NL:
                    # pull the Exp LUT load into the next layer's QKV phase so
                    # the first scores->exp of its attention doesn't stall
                    act_warmup(l, AF.Exp)

            # classifier head: logits[t, :] (cls_b added on host)
            for t in range(TT):
                pcl = ps.tile([128, S], F32, name=f"pcls{t}", tag="pp", bufs=2)
                for k in range(KT):
                    nc.tensor.matmul(
                        pcl[:, 0:NT],
                        h[k][:, t * 128 : (t + 1) * 128],
                        sb_cls[:, k * NT : (k + 1) * NT],
                        start=(k == 0),
                        stop=(k == KT - 1),
                    )
                ot = sb.tile([128, NT], F32, name=f"lg{t}", tag="lg", bufs=4)
                nc.scalar.activation(ot, pcl[:, 0:NT], AF.Copy)
                nc.sync.dma_start(logits[t * 128 : (t + 1) * 128, :], ot)

    nc.finalize()
    return nc


class _Runner:
    """Cached jitted SPMD executor (mirrors bass2jax.run_bass_via_pjrt)."""

    def __init__(self, nc, n_cores=8):
        import jax
        from jax.experimental.shard_map import shard_map
        from jax.sharding import Mesh, PartitionSpec
        from concourse import bass2jax, mybir as _mybir

        bass2jax.install_neuronx_cc_hook()
        self.n_cores = n_cores
        partition_name = (
            nc.partition_id_tensor.name if nc.partition_id_tensor else None
        )
        in_names, out_names, out_avals, zero_outs = [], [], [], []
        for alloc in nc.m.functions[0].allocations:
            if not isinstance(alloc, _mybir.MemoryLocationSet):
                continue
            name = alloc.memorylocations[0].name
            if alloc.kind == "ExternalInput":
                if name != partition_name:
                    in_names.append(name)
            elif alloc.kind == "ExternalOutput":
                shape = tuple(alloc.tensor_shape)
                dtype = _mybir.dt.np(alloc.dtype)
                out_names.append(name)
                out_avals.append(jax.core.ShapedArray(shape, dtype))
                zero_outs.append(np.zeros(shape, dtype))
        self.in_names = list(in_names)
        self.out_names = out_names
        self.out_avals = out_avals
        self.zero_outs = zero_outs
        n_params = len(in_names)
        n_outs = len(out_avals)
        donate = tuple(range(n_params, n_params + n_outs))
        all_in_names = tuple(
            in_names + out_names + ([partition_name] if partition_name else [])
        )

        def _body(*args):
            operands = list(args)
            if partition_name is not None:
                operands.append(bass2jax.partition_id_tensor())
            outs = bass2jax._bass_exec_p.bind(
                *operands,
                out_avals=tuple(out_avals),
                in_names=all_in_names,
                out_names=tuple(out_names),
                lowering_input_output_aliases=(),
                sim_require_finite=True,
                sim_require_nnan=True,
                nc=nc,
            )
            return tuple(outs)

        devices = jax.devices()[:n_cores]
        mesh = Mesh(np.asarray(devices), ("core",))
        in_specs = (PartitionSpec("core"),) * (n_params + n_outs)
        out_specs = (PartitionSpec("core"),) * n_outs
        self.sharded = jax.jit(
            shard_map(
                _body, mesh=mesh, in_specs=in_specs, out_specs=out_specs,
                check_rep=False,
            ),
            donate_argnums=donate,
            keep_unused=True,
        )

    def run(self, in_maps):
        nc_ = self.n_cores
        concat_in = [
            np.concatenate([np.asarray(m[name]) for m in in_maps], axis=0)
            for name in self.in_names
        ]
        concat_zeros = [
            np.zeros((nc_ * z.shape[0], *z.shape[1:]), z.dtype)
            for z in self.zero_outs
        ]
        out_arrs = self.sharded(*concat_in, *concat_zeros)
        return [
            {
                name: np.asarray(out_arrs[i]).reshape(
                    nc_, *self.out_avals[i].shape
                )[c]
                for i, name in enumerate(self.out_names)
            }
            for c in range(nc_)
        ]


_PACK_CACHE = {}
_PROG_CACHE = {}


def _pack_params(params):
    key = id(params.get("Wq", None))
    hit = _PACK_CACHE.get("k")
    if hit is not None and hit[0] == key:
        return hit[1], hit[2]
    p = {k: np.asarray(v) for k, v in params.items()}

    def as_blob(w, nk, cols):
        # [nk*128, cols] -> [128, nk*cols] with k-tile-major column order
        return np.ascontiguousarray(
            w.reshape(nk, 128, cols).transpose(1, 0, 2).reshape(128, nk * cols)
        ).astype(ml_dtypes.bfloat16)

    wa = np.stack(
        [
            np.concatenate(
                [as_blob(p[nm][l], KT, H) for nm in ("Wq", "Wk", "Wv", "Wo")], axis=1
            )
            for l in range(NL)
        ]
    )
    wb = np.stack([as_blob(p["Wi"][l], KT, FF) for l in range(NL)])
    wc = np.stack([as_blob(p["Wo2"][l], FKT, H) for l in range(NL)])

    def cols6(v):
        return v.reshape(-1, 128).T.astype(np.float32)  # [128, len/128]

    smallp = np.zeros((128, SPC), np.float32)
    smallp[:, 0:6] = cols6(p["emb_ln_g"])
    smallp[:, 6:12] = cols6(p["emb_ln_b"])
    for l in range(NL):
        b = 12 + SPL * l
        smallp[:, b : b + 6] = cols6(p["ln1_g"][l])
        smallp[:, b + 6 : b + 12] = cols6(p["ln1_b"][l])
        smallp[:, b + 12 : b + 18] = cols6(p["ln2_g"][l])
        smallp[:, b + 18 : b + 24] = cols6(p["ln2_b"][l])
        smallp[:, b + 24 : b + 30] = cols6(p["bq"][l])
        smallp[:, b + 30 : b + 36] = cols6(p["bk"][l])
        smallp[:, b + 36 : b + 42] = cols6(p["bo"][l])
        smallp[:, b + 42 : b + 48] = cols6(p["bo2"][l])
        smallp[:, b + 48 : b + 72] = cols6(p["bi"][l])
    smallb = (
        p["cls_W"].reshape(KT, 128, NT).transpose(1, 0, 2).reshape(128, KT * NT)
    ).astype(ml_dtypes.bfloat16)
    smallrow = p["bv"].reshape(1, NL * H).astype(np.float32)

    z = lambda a: bool(np.all(a == 0.0))
    flags = (
        z(p["bq"]), z(p["bk"]), z(p["bv"]), z(p["bo"]), z(p["bi"]), z(p["bo2"]),
        bool(
            np.all(p["ln1_g"] == 1) and np.all(p["ln2_g"] == 1)
            and np.all(p["emb_ln_g"] == 1) and z(p["ln1_b"]) and z(p["ln2_b"])
            and z(p["emb_ln_b"])
        ),
    )
    blobs = {
        "wA": wa, "wB": wb, "wC": wc,
        "smallp": smallp, "smallb": smallb, "smallrow": smallrow,
    }
    _PACK_CACHE["k"] = (key, blobs, flags)
    return blobs, flags


def _logsumexp(x, axis):
    m = np.max(x, axis=axis, keepdims=True)
    return np.squeeze(m, axis) + np.log(np.sum(np.exp(x - m), axis=axis))


def _crf_nll(params, logits, labels, mask):
    """torchcrf-style -sum(llh); mirrors the jax reference, in float64."""
    trans = np.asarray(params["crf_trans"], np.float64)
    cstart = np.asarray(params["crf_start"], np.float64)
    cend = np.asarray(params["crf_end"], np.float64)
    lg = logits.astype(np.float64)
    maskf = mask.astype(np.float64)
    Bb, Ss = labels.shape
    em_tag = np.take_along_axis(lg, labels[..., None], axis=2)[..., 0]
    trans_sc = trans[labels[:, :-1], labels[:, 1:]]
    num = cstart[labels[:, 0]] + em_tag[:, 0]
    num = num + np.sum(maskf[:, 1:] * (trans_sc + em_tag[:, 1:]), axis=1)
    seq_ends = mask.astype(np.int64).sum(1) - 1
    num = num + cend[labels[np.arange(Bb), seq_ends]]
    alpha = cstart[None] + lg[:, 0]
    for t in range(1, Ss):
        nxt = _logsumexp(alpha[:, :, None] + trans[None] + lg[:, t][:, None, :], 1)
        alpha = np.where(maskf[:, t][:, None] > 0, nxt, alpha)
    den = _logsumexp(alpha + cend[None], 1)
    return np.sum(den - num)


def _reference_numpy(input_ids, attention_mask, token_type_ids, labels, params):
    """Full-precision numpy fallback (used only if attention_mask has zeros,
    which the device kernel does not model in the attention bias)."""
    p = {k: np.asarray(v, np.float64) if np.asarray(v).dtype.kind == "f" else
         np.asarray(v) for k, v in params.items()}

    def ln(x, g, b):
        m = x.mean(-1, keepdims=True)
        v_ = ((x - m) ** 2).mean(-1, keepdims=True)
        return (x - m) / np.sqrt(v_ + EPS) * g + b

    x = p["word_emb"][input_ids] + p["pos_emb"][None, :S] + p["type_emb"][token_type_ids]
    x = ln(x, p["emb_ln_g"], p["emb_ln_b"])
    bias = np.where(attention_mask[:, None, None, :] > 0, 0.0, -1e9)
    from scipy.special import erf  # available in the image; exact gelu

    for l in range(NL):
        q = (x @ p["Wq"][l] + p["bq"][l]).reshape(B, S, NH, DH)
        k = (x @ p["Wk"][l] + p["bk"][l]).reshape(B, S, NH, DH)
        v_ = (x @ p["Wv"][l] + p["bv"][l]).reshape(B, S, NH, DH)
        sc = np.einsum("bqhd,bkhd->bhqk", q, k) / np.sqrt(DH) + bias
        sc = sc - sc.max(-1, keepdims=True)
        a = np.exp(sc)
        a /= a.sum(-1, keepdims=True)
        ctx = np.einsum("bhqk,bkhd->bqhd", a, v_).reshape(B, S, H)
        x = ln(x + ctx @ p["Wo"][l] + p["bo"][l], p["ln1_g"][l], p["ln1_b"][l])
        f = x @ p["Wi"][l] + p["bi"][l]
        f = 0.5 * f * (1.0 + erf(f / np.sqrt(2.0)))
        x = ln(x + f @ p["Wo2"][l] + p["bo2"][l], p["ln2_g"][l], p["ln2_b"][l])
    return (x @ p["cls_W"] + p["cls_b"]).astype(np.float32)


def _get_runner(flags):
    hit = _PROG_CACHE.get(flags)
    if hit is None:
        nc = _build_program(flags)
        hit = _Runner(nc, 8)
        _PROG_CACHE[flags] = hit
    return hit


def kernel(input_ids, attention_mask, token_type_ids, labels, params):
    ids = np.asarray(input_ids).astype(np.int64)
    tt = np.asarray(token_type_ids).astype(np.int64)
    mask = np.asarray(attention_mask)
    lab = np.asarray(labels).astype(np.int64)

    if not np.all(np.asarray(mask) > 0):
        lg = _reference_numpy(ids, np.asarray(mask), tt, lab, params)
        return np.asarray(_crf_nll(params, lg, lab, mask), np.float32)

    blobs, flags = _pack_params(params)
    runner = _get_runner(flags)

    we = np.asarray(params["word_emb"], np.float32)
    pe = np.asarray(params["pos_emb"], np.float32)
    te = np.asarray(params["type_emb"], np.float32)
    x0 = we[ids] + pe[None, :S] + te[tt]  # [B, S, H] fp32
    x0T = np.ascontiguousarray(x0.transpose(0, 2, 1)).astype(ml_dtypes.bfloat16)

    in_maps = []
    for c in range(B):
        m = {"x0T": x0T[c]}
        for name in runner.in_names:
            if name != "x0T":
                m[name] = blobs[name]
        in_maps.append(m)
    res = runner.run(in_maps)
    logits = np.stack([res[c]["logits"] for c in range(B)]).astype(np.float32)
    logits = logits + np.asarray(params["cls_b"], np.float32)[None, None, :]
    return np.asarray(_crf_nll(params, logits, lab, mask), np.float32)


# revision 33
# speedup vs baseline: 1.0696x; 1.0696x over previous
"""BERT-base + CRF loss kernel for 8 Trainium2 NeuronCores.

Strategy (data-parallel over batch, B=8 -> 1 sequence per core):
  - Host: embedding gather (+pos/+type), pack weights into bf16 DMA blobs.
  - Device (per core): 12-layer BERT encoder + classifier head on one
    [512, 768] sequence, activations kept in transposed [H, T] layout
    (hidden on partitions) so no transposes are ever needed. Attention
    computes exp(S^T) unnormalized; a ones-column baked into V makes the
    softmax denominator fall out of the AV matmul; normalization is fused
    into the ctx eviction. Matmuls in bf16 (1 col/cycle), fp32 PSUM.
  - Host: CRF negative log-likelihood on the [B, 512, 9] emissions
    (tiny sequential scan, bad fit for the PE) and the final scalar sum.
"""

import os
import sys

for _p in ("/opt/trn_rl_repo", "/root/.axon_site/_ro/trn_rl_repo"):
    if os.path.isdir(_p) and _p not in sys.path:
        sys.path.insert(0, _p)

import numpy as np
import ml_dtypes

import concourse.bass as bass
import concourse.tile as tile
from concourse import bacc, mybir

BF16 = mybir.dt.bfloat16
F32 = mybir.dt.float32
AF = mybir.ActivationFunctionType
OP = mybir.AluOpType

V, H, NL, NH, DH, FF, MAXPOS, NT = 30522, 768, 12, 12, 64, 3072, 512, 9
B, S = 8, 512
EPS = 1e-12
KT = H // 128        # 6 hidden k-tiles
FKT = FF // 128      # 24 ffn k-tiles
TT = S // 128        # 4 token tiles
WAC = 18432          # blob cols per layer
HALF = WAC // 2      # 9216
SPL = 72             # smallp cols per layer
SPC = 12 + SPL * NL  # smallp total cols


def _build_program(flags):
    """flags = (zq, zk, zv, zo, zi, zo2, ln_triv); z* True means that bias is
    all-zero (skip adding it), ln_triv True means every LN gamma==1, beta==0."""
    zq, zk, zv, zo, zi, zo2, ln_triv = flags
    nc = bacc.Bacc("TRN2", target_bir_lowering=False, debug=False)

    x0T = nc.dram_tensor("x0T", [H, S], BF16, kind="ExternalInput")
    wA = nc.dram_tensor("wA", [NL, 128, WAC], BF16, kind="ExternalInput")
    wB = nc.dram_tensor("wB", [NL, 128, WAC], BF16, kind="ExternalInput")
    wC = nc.dram_tensor("wC", [NL, 128, WAC], BF16, kind="ExternalInput")
    need_smallp = (not ln_triv) or not (zq and zk and zo and zi and zo2)
    smallp = (
        nc.dram_tensor("smallp", [128, SPC], F32, kind="ExternalInput")
        if need_smallp
        else None
    )
    smallb = nc.dram_tensor("smallb", [128, KT * NT], BF16, kind="ExternalInput")
    smallrow = (
        nc.dram_tensor("smallrow", [1, NL * H], F32, kind="ExternalInput")
        if not zv
        else None
    )
    logits = nc.dram_tensor("logits", [S, NT], F32, kind="ExternalOutput")

    with tile.TileContext(nc) as tc:
        with (
            tc.tile_pool(name="sb", bufs=1) as sb,
            tc.tile_pool(name="ps", bufs=1, space="PSUM") as ps,
        ):
            ones1 = sb.tile([128, 1], BF16, name="ones1")
            nc.vector.memset(ones1, 1.0)
            onesr = sb.tile([1, 128], F32, name="onesr")
            nc.vector.memset(onesr, 1.0)
            eps_sc = sb.tile([1, 1], F32, name="eps_sc")
            nc.vector.memset(eps_sc, EPS)
            warm_sc = sb.tile([1, 1], F32, name="warm_sc")
            nc.vector.memset(warm_sc, 1.0)

            def act_warmup(i, func):
                # touch an ACT LUT while the PE is busy elsewhere so the
                # 1.3us table load doesn't land on a critical path later
                w1 = sb.tile([1, 1], F32, name=f"aw{i}", tag="warm", bufs=2)
                nc.scalar.activation(w1, warm_sc, func)
            sp_sb = None
            if smallp is not None:
                sp_sb = sb.tile([128, SPC], F32, name="sp_sb")
                nc.sync.dma_start(sp_sb, smallp[:, :])
            sb_cls = sb.tile([128, KT * NT], BF16, name="sb_cls")
            nc.sync.dma_start(sb_cls, smallb[:, :])
            srow_sb = None
            if smallrow is not None:
                srow_sb = sb.tile([1, NL * H], F32, name="srow_sb")
                nc.sync.dma_start(srow_sb, smallrow[:, :])

            def pcol(off):
                assert sp_sb is not None
                return sp_sb[:, off : off + 1]

            row_i = [0]

            def rowtile(nm):
                row_i[0] += 1
                return sb.tile([1, S], F32, name=f"r{nm}{row_i[0]}", tag="row", bufs=6)

            ln_i = [0]

            def layer_norm(r_tiles, goff, boff):
                """r_tiles: 6 bf16 [128, S] tiles. Returns 6 new bf16 tiles.

                Critical-path layout: the -mean broadcast happens as soon as
                the plain sum is done, so the centering pass (pass 1) for
                every tile overlaps the sum-of-squares / rstd chain; only the
                final multiply by rstd (pass 2) waits for the full chain.
                Broadcasts are ones-matmuls on the PE (keeps HAM warm during
                the LN bubble)."""
                ln_i[0] += 1
                i = ln_i[0]
                s_ps = ps.tile([1, S], F32, name=f"lns{i}", tag="psrow", bufs=1)
                s2_ps = ps.tile([1, S], F32, name=f"lns2{i}", tag="pscore", bufs=3)
                for k in range(KT):
                    nc.tensor.matmul(
                        s_ps, ones1, r_tiles[k], start=(k == 0), stop=(k == KT - 1)
                    )
                # -mean row + its broadcast (early; feeds pass 1). The
                # broadcast is a K=1 fp32 ones-matmul into PSUM: exact, and it
                # is real PE activity in the middle of the LN bubble.
                mb_row = rowtile("mb")
                nc.vector.tensor_scalar_mul(mb_row, s_ps, -1.0 / H)
                mb_b = sb.tile([128, S], F32, name=f"lnmb{i}", tag="bc", bufs=4)
                nc.gpsimd.partition_broadcast(mb_b, mb_row)
                # sum of squares -> rstd
                for k in range(KT):
                    sq = sb.tile([128, S], BF16, name=f"lnsq{i}_{k}", tag="xsq", bufs=2)
                    nc.scalar.activation(sq, r_tiles[k], AF.Square)
                    nc.tensor.matmul(
                        s2_ps, ones1, sq, start=(k == 0), stop=(k == KT - 1)
                    )
                mm_row = rowtile("mm")  # mb^2 = (s/H)^2
                nc.vector.tensor_mul(mm_row, mb_row, mb_row)
                u_row = rowtile("u")  # u = s2 - H*mb^2  (var = u/H)
                nc.vector.scalar_tensor_tensor(
                    u_row, mm_row, -float(H), s2_ps, op0=OP.mult, op1=OP.add
                )
                std_row = rowtile("std")  # sqrt(u/H + eps)
                nc.scalar.activation(
                    std_row, u_row, AF.Sqrt, bias=eps_sc[:, :], scale=1.0 / H
                )
                rr_row = rowtile("rr")
                nc.vector.reciprocal_approx_fast(rr_row, std_row)
                rb_b = sb.tile([128, S], F32, name=f"lnrb{i}", tag="bc", bufs=4)
                nc.gpsimd.partition_broadcast(rb_b, rr_row)
                # dummy weight loads: PE activity pulses spaced through the LN
                # bubble (dep on successive row tiles of the rstd chain) so
                # the HAM monitor never demotes the clock. Values are unused —
                # every matmul issues its own LDWEIGHTS.
                for src in (u_row, std_row, rr_row):
                    v = src.bitcast(BF16)
                    for b in range(3):
                        nc.tensor.ldweights(v[:1, b * 128 : (b + 1) * 128])
                out = []
                for k in range(KT):
                    t1 = sb.tile([128, S], F32, name=f"lnt{i}_{k}", tag="lnt", bufs=6)
                    nc.vector.tensor_add(t1, r_tiles[k], mb_b)  # pass 1 (early)
                    o = sb.tile([128, S], BF16, name=f"lno{i}_{k}", tag="h", bufs=16)
                    if ln_triv:
                        nc.vector.tensor_mul(o, t1, rb_b)  # pass 2
                    else:
                        t2 = sb.tile(
                            [128, S], F32, name=f"lnu{i}_{k}", tag="lnt2", bufs=2
                        )
                        nc.vector.tensor_mul(t2, t1, rb_b)
                        nc.vector.tensor_scalar(
                            o, t2, pcol(goff + k), pcol(boff + k),
                            op0=OP.mult, op1=OP.add,
                        )
                    out.append(o)
                return out

            # ---- embeddings LN ----
            x0 = []
            for k in range(KT):
                xt = sb.tile([128, S], BF16, name=f"x0_{k}", tag="h", bufs=16)
                nc.sync.dma_start(xt, x0T[k * 128 : (k + 1) * 128, :])
                x0.append(xt)
            h = layer_norm(x0, 0, 6)
            act_warmup(99, AF.Exp)

            for l in range(NL):
                base = 12 + SPL * l
                wa1 = sb.tile([128, HALF], BF16, name=f"wa1_{l}", tag="w", bufs=4)
                nc.sync.dma_start(wa1, wA[l, :, 0:HALF])
                wa2 = sb.tile([128, HALF], BF16, name=f"wa2_{l}", tag="w", bufs=4)
                nc.sync.dma_start(wa2, wA[l, :, HALF:WAC])

                # V first (token-major with a ones column per head:
                # v_sb[t] is [128 tokens, 12 heads, 65], col 64 == 1.0)
                v_sb = []
                for t in range(TT):
                    vt = sb.tile([128, NH, DH + 1], BF16, name=f"v{l}_{t}", tag="v",
                                 bufs=5)
                    nc.vector.memset(vt[:, :, DH : DH + 1], 1.0)
                    v_sb.append(vt)
                for t in range(TT):
                    for n in range(2):
                        pv = ps.tile(
                            [128, 384], F32, name=f"pv{l}_{t}_{n}", tag="pp", bufs=2
                        )
                        for k in range(KT):
                            nc.tensor.matmul(
                                pv,
                                h[k][:, t * 128 : (t + 1) * 128],
                                wa2[:, k * H + n * 384 : k * H + (n + 1) * 384],
                                start=(k == 0),
                                stop=(k == KT - 1),
                            )
                        dstv = v_sb[t][:, n * 6 : (n + 1) * 6, 0:DH]
                        pv3 = pv.rearrange("p (a b) -> p a b", a=6)
                        if zv:
                            nc.vector.tensor_copy(dstv, pv3)
                        else:
                            assert srow_sb is not None
                            bvb = sb.tile(
                                [128, 384], F32, name=f"bvb{l}_{t}_{n}", tag="bvb",
                                bufs=2,
                            )
                            nc.gpsimd.partition_broadcast(
                                bvb, srow_sb[:, l * H + n * 384 : l * H + (n + 1) * 384]
                            )
                            nc.vector.tensor_add(
                                dstv, pv3, bvb.rearrange("p (a b) -> p a b", a=6)
                            )

                # Q^T/K^T projections software-pipelined INTO the attention
                # head loop: the projection matmuls for h-tile m+1 are emitted
                # between the scores and AV of head pair m, so the in-order PE
                # stream has dense work while ACT chases the 48 exp evictions
                # (attention alone is exp-bound on ACT).
                def qk_tile(m):
                    pair = []
                    for which, woff, bias_off, bz in (
                        (0, 0, base + 24, zq),
                        (1, 4608, base + 30, zk),
                    ):
                        pq = ps.tile(
                            [128, S], F32, name=f"pq{l}_{which}_{m}", tag="pp", bufs=2
                        )
                        for k in range(KT):
                            nc.tensor.matmul(
                                pq,
                                wa1[:, woff + k * H + m * 128 : woff + k * H + (m + 1) * 128],
                                h[k],
                                start=(k == 0),
                                stop=(k == KT - 1),
                            )
                        o = sb.tile(
                            [128, S], BF16, name=f"qk{l}_{which}_{m}", tag="qkt",
                            bufs=6,
                        )
                        if bz:
                            nc.vector.tensor_copy(o, pq)
                        else:
                            nc.vector.tensor_scalar_add(o, pq, pcol(bias_off + m))
                        pair.append(o)
                    return pair

                ctxt = [
                    sb.tile([128, S], BF16, name=f"ctx{l}_{k}", tag="ctx", bufs=8)
                    for k in range(KT)
                ]

                def head_scores(hd, qt_m, kt_m):
                    poff = (hd % 2) * DH
                    kt_sl = kt_m[poff : poff + DH, :]
                    qt_sl = qt_m[poff : poff + DH, :]
                    e_tiles = []
                    for t4 in range(TT):
                        pss = ps.tile(
                            [128, S], F32, name=f"psS{l}_{hd}_{t4}", tag="pscore",
                            bufs=3,
                        )
                        nc.tensor.matmul(
                            pss,
                            kt_sl[:, t4 * 128 : (t4 + 1) * 128],
                            qt_sl,
                            start=True,
                            stop=True,
                        )
                        et = sb.tile(
                            [128, S], BF16, name=f"e{l}_{hd}_{t4}", tag="e", bufs=10
                        )
                        nc.scalar.activation(et, pss, AF.Exp, scale=0.125)
                        e_tiles.append(et)
                    return e_tiles

                def head_av(hd, e_tiles):
                    kti = hd // 2
                    poff = (hd % 2) * DH
                    pc = ps.tile([DH + 1, S], F32, name=f"pc{l}_{hd}", tag="pctx",
                                 bufs=2)
                    for t4 in range(TT):
                        nc.tensor.matmul(
                            pc,
                            v_sb[t4][:, hd, :],
                            e_tiles[t4],
                            start=(t4 == 0),
                            stop=(t4 == TT - 1),
                        )
                    # free the PSUM bank fast: evict unnormalized ctx via ACT,
                    # then normalize off the PE/PSUM critical path on DVE.
                    # (reciprocal_approx_fast drops nonzero partition offsets,
                    # so stage Z to a partition-0 tile first)
                    zz = rowtile("zz")
                    nc.scalar.activation(zz, pc[DH : DH + 1, :], AF.Copy)
                    rz = rowtile("rz")
                    nc.vector.reciprocal_approx_fast(rz, zz)
                    cu = sb.tile([DH, S], F32, name=f"cu{l}_{hd}", tag="cu", bufs=4)
                    nc.scalar.activation(cu, pc[0:DH, :], AF.Copy)
                    rzb = sb.tile([DH, S], F32, name=f"rzb{l}_{hd}", tag="rzb",
                                  bufs=3)
                    nc.gpsimd.partition_broadcast(rzb, rz)
                    nc.vector.tensor_mul(ctxt[kti][poff : poff + DH, :], cu, rzb)

                qk_cur = qk_tile(0)
                for m in range(KT):
                    e0 = head_scores(2 * m, qk_cur[0], qk_cur[1])
                    e1 = head_scores(2 * m + 1, qk_cur[0], qk_cur[1])
                    if m + 1 < KT:
                        qk_next = qk_tile(m + 1)  # PE filler while exps run
                    head_av(2 * m, e0)
                    head_av(2 * m + 1, e1)
                    if m + 1 < KT:
                        qk_cur = qk_next

                # Wo projection + residual
                r1 = []
                for m in range(KT):
                    po = ps.tile([128, S], F32, name=f"po{l}_{m}", tag="pp", bufs=2)
                    for k in range(KT):
                        nc.tensor.matmul(
                            po,
                            wa2[:, 4608 + k * H + m * 128 : 4608 + k * H + (m + 1) * 128],
                            ctxt[k],
                            start=(k == 0),
                            stop=(k == KT - 1),
                        )
                    o = sb.tile([128, S], BF16, name=f"r1_{l}_{m}", tag="h", bufs=16)
                    if zo:
                        nc.vector.tensor_add(o, h[m], po)
                    else:
                        nc.vector.scalar_tensor_tensor(
                            o, po, pcol(base + 36 + m), h[m], op0=OP.add, op1=OP.add
                        )
                    r1.append(o)
                h1 = layer_norm(r1, base + 0, base + 6)

                # FFN
                wb1 = sb.tile([128, HALF], BF16, name=f"wb1_{l}", tag="w", bufs=4)
                nc.sync.dma_start(wb1, wB[l, :, 0:HALF])
                wb2 = sb.tile([128, HALF], BF16, name=f"wb2_{l}", tag="w", bufs=4)
                nc.sync.dma_start(wb2, wB[l, :, HALF:WAC])
                g = []
                for fm in range(FKT):
                    pg = ps.tile([128, S], F32, name=f"pg{l}_{fm}", tag="pp", bufs=2)
                    for k in range(KT):
                        wh = wb1 if k < 3 else wb2
                        off = (k % 3) * FF + fm * 128
                        nc.tensor.matmul(
                            pg, wh[:, off : off + 128], h1[k],
                            start=(k == 0), stop=(k == KT - 1),
                        )
                    go = sb.tile([128, S], BF16, name=f"g{l}_{fm}", tag="g", bufs=24)
                    if zi:
                        nc.scalar.activation(go, pg, AF.Gelu)
                    else:
                        nc.scalar.activation(go, pg, AF.Gelu, bias=pcol(base + 48 + fm))
                    g.append(go)

                wc1 = sb.tile([128, HALF], BF16, name=f"wc1_{l}", tag="w", bufs=4)
                nc.sync.dma_start(wc1, wC[l, :, 0:HALF])
                wc2 = sb.tile([128, HALF], BF16, name=f"wc2_{l}", tag="w", bufs=4)
                nc.sync.dma_start(wc2, wC[l, :, HALF:WAC])
                r2 = []
                for m in range(KT):
                    pf = ps.tile([128, S], F32, name=f"pf{l}_{m}", tag="pp", bufs=2)
                    for k in range(FKT):
                        wh = wc1 if k < 12 else wc2
                        off = (k % 12) * H + m * 128
                        nc.tensor.matmul(
                            pf, wh[:, off : off + 128], g[k],
                            start=(k == 0), stop=(k == FKT - 1),
                        )
                    o = sb.tile([128, S], BF16, name=f"r2_{l}_{m}", tag="h", bufs=16)
                    if zo2:
                        nc.vector.tensor_add(o, h1[m], pf)
                    else:
                        nc.vector.scalar_tensor_tensor(
                            o, pf, pcol(base + 42 + m), h1[m], op0=OP.add, op1=OP.add
                        )
                    r2.append(o)
                h = layer_norm(r2, base + 12, base + 18)
                if l + 1 < NL:
                    # pull the Exp LUT load into the next layer's QKV phase so
                    # the first scores->exp of its attention doesn't stall
                    act_warmup(l, AF.Exp)

            # classifier head: logits[t, :] (cls_b added on host)
            for t in range(TT):
                pcl = ps.tile([128, S], F32, name=f"pcls{t}", tag="pp", bufs=2)
                for k in range(KT):
                    nc.tensor.matmul(
                        pcl[:, 0:NT],
                        h[k][:, t * 128 : (t + 1) * 128],
                        sb_cls[:, k * NT : (k + 1) * NT],
                        start=(k == 0),
                        stop=(k == KT - 1),
                    )
                ot = sb.tile([128, NT], F32, name=f"lg{t}", tag="lg", bufs=4)
                nc.scalar.activation(ot, pcl[:, 0:NT], AF.Copy)
                nc.sync.dma_start(logits[t * 128 : (t + 1) * 128, :], ot)

    nc.finalize()
    return nc


class _Runner:
    """Cached jitted SPMD executor (mirrors bass2jax.run_bass_via_pjrt)."""

    def __init__(self, nc, n_cores=8):
        import jax
        from jax.experimental.shard_map import shard_map
        from jax.sharding import Mesh, PartitionSpec
        from concourse import bass2jax, mybir as _mybir

        bass2jax.install_neuronx_cc_hook()
        self.n_cores = n_cores
        partition_name = (
            nc.partition_id_tensor.name if nc.partition_id_tensor else None
        )
        in_names, out_names, out_avals, zero_outs = [], [], [], []
        for alloc in nc.m.functions[0].allocations:
            if not isinstance(alloc, _mybir.MemoryLocationSet):
                continue
            name = alloc.memorylocations[0].name
            if alloc.kind == "ExternalInput":
                if name != partition_name:
                    in_names.append(name)
            elif alloc.kind == "ExternalOutput":
                shape = tuple(alloc.tensor_shape)
                dtype = _mybir.dt.np(alloc.dtype)
                out_names.append(name)
                out_avals.append(jax.core.ShapedArray(shape, dtype))
                zero_outs.append(np.zeros(shape, dtype))
        self.in_names = list(in_names)
        self.out_names = out_names
        self.out_avals = out_avals
        self.zero_outs = zero_outs
        n_params = len(in_names)
        n_outs = len(out_avals)
        donate = tuple(range(n_params, n_params + n_outs))
        all_in_names = tuple(
            in_names + out_names + ([partition_name] if partition_name else [])
        )

        def _body(*args):
            operands = list(args)
            if partition_name is not None:
                operands.append(bass2jax.partition_id_tensor())
            outs = bass2jax._bass_exec_p.bind(
                *operands,
                out_avals=tuple(out_avals),
                in_names=all_in_names,
                out_names=tuple(out_names),
                lowering_input_output_aliases=(),
                sim_require_finite=True,
                sim_require_nnan=True,
                nc=nc,
            )
            return tuple(outs)

        devices = jax.devices()[:n_cores]
        mesh = Mesh(np.asarray(devices), ("core",))
        in_specs = (PartitionSpec("core"),) * (n_params + n_outs)
        out_specs = (PartitionSpec("core"),) * n_outs
        self.sharded = jax.jit(
            shard_map(
                _body, mesh=mesh, in_specs=in_specs, out_specs=out_specs,
                check_rep=False,
            ),
            donate_argnums=donate,
            keep_unused=True,
        )

    def run(self, in_maps):
        nc_ = self.n_cores
        concat_in = [
            np.concatenate([np.asarray(m[name]) for m in in_maps], axis=0)
            for name in self.in_names
        ]
        concat_zeros = [
            np.zeros((nc_ * z.shape[0], *z.shape[1:]), z.dtype)
            for z in self.zero_outs
        ]
        out_arrs = self.sharded(*concat_in, *concat_zeros)
        return [
            {
                name: np.asarray(out_arrs[i]).reshape(
                    nc_, *self.out_avals[i].shape
                )[c]
                for i, name in enumerate(self.out_names)
            }
            for c in range(nc_)
        ]


_PACK_CACHE = {}
_PROG_CACHE = {}


def _pack_params(params):
    key = id(params.get("Wq", None))
    hit = _PACK_CACHE.get("k")
    if hit is not None and hit[0] == key:
        return hit[1], hit[2]
    p = {k: np.asarray(v) for k, v in params.items()}

    def as_blob(w, nk, cols):
        # [nk*128, cols] -> [128, nk*cols] with k-tile-major column order
        return np.ascontiguousarray(
            w.reshape(nk, 128, cols).transpose(1, 0, 2).reshape(128, nk * cols)
        ).astype(ml_dtypes.bfloat16)

    wa = np.stack(
        [
            np.concatenate(
                [as_blob(p[nm][l], KT, H) for nm in ("Wq", "Wk", "Wv", "Wo")], axis=1
            )
            for l in range(NL)
        ]
    )
    wb = np.stack([as_blob(p["Wi"][l], KT, FF) for l in range(NL)])
    wc = np.stack([as_blob(p["Wo2"][l], FKT, H) for l in range(NL)])

    def cols6(v):
        return v.reshape(-1, 128).T.astype(np.float32)  # [128, len/128]

    smallp = np.zeros((128, SPC), np.float32)
    smallp[:, 0:6] = cols6(p["emb_ln_g"])
    smallp[:, 6:12] = cols6(p["emb_ln_b"])
    for l in range(NL):
        b = 12 + SPL * l
        smallp[:, b : b + 6] = cols6(p["ln1_g"][l])
        smallp[:, b + 6 : b + 12] = cols6(p["ln1_b"][l])
        smallp[:, b + 12 : b + 18] = cols6(p["ln2_g"][l])
        smallp[:, b + 18 : b + 24] = cols6(p["ln2_b"][l])
        smallp[:, b + 24 : b + 30] = cols6(p["bq"][l])
        smallp[:, b + 30 : b + 36] = cols6(p["bk"][l])
        smallp[:, b + 36 : b + 42] = cols6(p["bo"][l])
        smallp[:, b + 42 : b + 48] = cols6(p["bo2"][l])
        smallp[:, b + 48 : b + 72] = cols6(p["bi"][l])
    smallb = (
        p["cls_W"].reshape(KT, 128, NT).transpose(1, 0, 2).reshape(128, KT * NT)
    ).astype(ml_dtypes.bfloat16)
    smallrow = p["bv"].reshape(1, NL * H).astype(np.float32)

    z = lambda a: bool(np.all(a == 0.0))
    flags = (
        z(p["bq"]), z(p["bk"]), z(p["bv"]), z(p["bo"]), z(p["bi"]), z(p["bo2"]),
        bool(
            np.all(p["ln1_g"] == 1) and np.all(p["ln2_g"] == 1)
            and np.all(p["emb_ln_g"] == 1) and z(p["ln1_b"]) and z(p["ln2_b"])
            and z(p["emb_ln_b"])
        ),
    )
    blobs = {
        "wA": wa, "wB": wb, "wC": wc,
        "smallp": smallp, "smallb": smallb, "smallrow": smallrow,
    }
    _PACK_CACHE["k"] = (key, blobs, flags)
    return blobs, flags


def _logsumexp(x, axis):
    m = np.max(x, axis=axis, keepdims=True)
    return np.squeeze(m, axis) + np.log(np.sum(np.exp(x - m), axis=axis))


def _crf_nll(params, logits, labels, mask):
    """torchcrf-style -sum(llh); mirrors the jax reference, in float64."""
    trans = np.asarray(params["crf_trans"], np.float64)
    cstart = np.asarray(params["crf_start"], np.float64)
    cend = np.asarray(params["crf_end"], np.float64)
    lg = logits.astype(np.float64)
    maskf = mask.astype(np.float64)
    Bb, Ss = labels.shape
    em_tag = np.take_along_axis(lg, labels[..., None], axis=2)[..., 0]
    trans_sc = trans[labels[:, :-1], labels[:, 1:]]
    num = cstart[labels[:, 0]] + em_tag[:, 0]
    num = num + np.sum(maskf[:, 1:] * (trans_sc + em_tag[:, 1:]), axis=1)
    seq_ends = mask.astype(np.int64).sum(1) - 1
    num = num + cend[labels[np.arange(Bb), seq_ends]]
    alpha = cstart[None] + lg[:, 0]
    for t in range(1, Ss):
        nxt = _logsumexp(alpha[:, :, None] + trans[None] + lg[:, t][:, None, :], 1)
        alpha = np.where(maskf[:, t][:, None] > 0, nxt, alpha)
    den = _logsumexp(alpha + cend[None], 1)
    return np.sum(den - num)


def _reference_numpy(input_ids, attention_mask, token_type_ids, labels, params):
    """Full-precision numpy fallback (used only if attention_mask has zeros,
    which the device kernel does not model in the attention bias)."""
    p = {k: np.asarray(v, np.float64) if np.asarray(v).dtype.kind == "f" else
         np.asarray(v) for k, v in params.items()}

    def ln(x, g, b):
        m = x.mean(-1, keepdims=True)
        v_ = ((x - m) ** 2).mean(-1, keepdims=True)
        return (x - m) / np.sqrt(v_ + EPS) * g + b

    x = p["word_emb"][input_ids] + p["pos_emb"][None, :S] + p["type_emb"][token_type_ids]
    x = ln(x, p["emb_ln_g"], p["emb_ln_b"])
    bias = np.where(attention_mask[:, None, None, :] > 0, 0.0, -1e9)
    from scipy.special import erf  # available in the image; exact gelu

    for l in range(NL):
        q = (x @ p["Wq"][l] + p["bq"][l]).reshape(B, S, NH, DH)
        k = (x @ p["Wk"][l] + p["bk"][l]).reshape(B, S, NH, DH)
        v_ = (x @ p["Wv"][l] + p["bv"][l]).reshape(B, S, NH, DH)
        sc = np.einsum("bqhd,bkhd->bhqk", q, k) / np.sqrt(DH) + bias
        sc = sc - sc.max(-1, keepdims=True)
        a = np.exp(sc)
        a /= a.sum(-1, keepdims=True)
        ctx = np.einsum("bhqk,bkhd->bqhd", a, v_).reshape(B, S, H)
        x = ln(x + ctx @ p["Wo"][l] + p["bo"][l], p["ln1_g"][l], p["ln1_b"][l])
        f = x @ p["Wi"][l] + p["bi"][l]
        f = 0.5 * f * (1.0 + erf(f / np.sqrt(2.0)))
        x = ln(x + f @ p["Wo2"][l] + p["bo2"][l], p["ln2_g"][l], p["ln2_b"][l])
    return (x @ p["cls_W"] + p["cls_b"]).astype(np.float32)


def _get_runner(flags):
    hit = _PROG_CACHE.get(flags)
    if hit is None:
        nc = _build_program(flags)
        hit = _Runner(nc, 8)
        _PROG_CACHE[flags] = hit
    return hit


def kernel(input_ids, attention_mask, token_type_ids, labels, params):
    ids = np.asarray(input_ids).astype(np.int64)
    tt = np.asarray(token_type_ids).astype(np.int64)
    mask = np.asarray(attention_mask)
    lab = np.asarray(labels).astype(np.int64)

    if not np.all(np.asarray(mask) > 0):
        lg = _reference_numpy(ids, np.asarray(mask), tt, lab, params)
        return np.asarray(_crf_nll(params, lg, lab, mask), np.float32)

    blobs, flags = _pack_params(params)
    runner = _get_runner(flags)

    we = np.asarray(params["word_emb"], np.float32)
    pe = np.asarray(params["pos_emb"], np.float32)
    te = np.asarray(params["type_emb"], np.float32)
    x0 = we[ids] + pe[None, :S] + te[tt]  # [B, S, H] fp32
    x0T = np.ascontiguousarray(x0.transpose(0, 2, 1)).astype(ml_dtypes.bfloat16)

    in_maps = []
    for c in range(B):
        m = {"x0T": x0T[c]}
        for name in runner.in_names:
            if name != "x0T":
                m[name] = blobs[name]
        in_maps.append(m)
    res = runner.run(in_maps)
    logits = np.stack([res[c]["logits"] for c in range(B)]).astype(np.float32)
    logits = logits + np.asarray(params["cls_b"], np.float32)[None, None, :]
    return np.asarray(_crf_nll(params, logits, lab, mask), np.float32)


# revision 34
# speedup vs baseline: 1.1219x; 1.0489x over previous
"""BERT-base + CRF loss kernel for 8 Trainium2 NeuronCores.

Strategy (data-parallel over batch, B=8 -> 1 sequence per core):
  - Host: embedding gather (+pos/+type), pack weights into bf16 DMA blobs.
  - Device (per core): 12-layer BERT encoder + classifier head on one
    [512, 768] sequence, activations kept in transposed [H, T] layout
    (hidden on partitions) so no transposes are ever needed. Attention
    computes exp(S^T) unnormalized; a ones-column baked into V makes the
    softmax denominator fall out of the AV matmul; normalization is fused
    into the ctx eviction. Matmuls in bf16 (1 col/cycle), fp32 PSUM.
  - Host: CRF negative log-likelihood on the [B, 512, 9] emissions
    (tiny sequential scan, bad fit for the PE) and the final scalar sum.
"""

import os
import sys

for _p in ("/opt/trn_rl_repo", "/root/.axon_site/_ro/trn_rl_repo"):
    if os.path.isdir(_p) and _p not in sys.path:
        sys.path.insert(0, _p)

import numpy as np
import ml_dtypes

import concourse.bass as bass
import concourse.tile as tile
from concourse import bacc, mybir

BF16 = mybir.dt.bfloat16
F32 = mybir.dt.float32
AF = mybir.ActivationFunctionType
OP = mybir.AluOpType

V, H, NL, NH, DH, FF, MAXPOS, NT = 30522, 768, 12, 12, 64, 3072, 512, 9
B, S = 8, 512
EPS = 1e-12
KT = H // 128        # 6 hidden k-tiles
FKT = FF // 128      # 24 ffn k-tiles
TT = S // 128        # 4 token tiles
WAC = 18432          # blob cols per layer
HALF = WAC // 2      # 9216
SPL = 72             # smallp cols per layer
SPC = 12 + SPL * NL  # smallp total cols


def _build_program(flags):
    """flags = (zq, zk, zv, zo, zi, zo2, ln_triv); z* True means that bias is
    all-zero (skip adding it), ln_triv True means every LN gamma==1, beta==0."""
    zq, zk, zv, zo, zi, zo2, ln_triv = flags
    nc = bacc.Bacc("TRN2", target_bir_lowering=False, debug=False)

    x0T = nc.dram_tensor("x0T", [H, S], BF16, kind="ExternalInput")
    wA = nc.dram_tensor("wA", [NL, 128, WAC], BF16, kind="ExternalInput")
    wB = nc.dram_tensor("wB", [NL, 128, WAC], BF16, kind="ExternalInput")
    wC = nc.dram_tensor("wC", [NL, 128, WAC], BF16, kind="ExternalInput")
    need_smallp = (not ln_triv) or not (zq and zk and zo and zi and zo2)
    smallp = (
        nc.dram_tensor("smallp", [128, SPC], F32, kind="ExternalInput")
        if need_smallp
        else None
    )
    smallb = nc.dram_tensor("smallb", [128, KT * NT], BF16, kind="ExternalInput")
    smallrow = (
        nc.dram_tensor("smallrow", [1, NL * H], F32, kind="ExternalInput")
        if not zv
        else None
    )
    logits = nc.dram_tensor("logits", [S, NT], F32, kind="ExternalOutput")

    with tile.TileContext(nc) as tc:
        with (
            tc.tile_pool(name="sb", bufs=1) as sb,
            tc.tile_pool(name="ps", bufs=1, space="PSUM") as ps,
        ):
            ones1 = sb.tile([128, 1], BF16, name="ones1")
            nc.vector.memset(ones1, 1.0)
            onesr = sb.tile([1, 128], F32, name="onesr")
            nc.vector.memset(onesr, 1.0)
            eps_sc = sb.tile([1, 1], F32, name="eps_sc")
            nc.vector.memset(eps_sc, EPS)
            warm_sc = sb.tile([1, 1], F32, name="warm_sc")
            nc.vector.memset(warm_sc, 1.0)

            def act_warmup(i, func):
                # touch an ACT LUT while the PE is busy elsewhere so the
                # 1.3us table load doesn't land on a critical path later
                w1 = sb.tile([1, 1], F32, name=f"aw{i}", tag="warm", bufs=2)
                nc.scalar.activation(w1, warm_sc, func)
            sp_sb = None
            if smallp is not None:
                sp_sb = sb.tile([128, SPC], F32, name="sp_sb")
                nc.sync.dma_start(sp_sb, smallp[:, :])
            sb_cls = sb.tile([128, KT * NT], BF16, name="sb_cls")
            nc.sync.dma_start(sb_cls, smallb[:, :])
            srow_sb = None
            if smallrow is not None:
                srow_sb = sb.tile([1, NL * H], F32, name="srow_sb")
                nc.sync.dma_start(srow_sb, smallrow[:, :])

            def pcol(off):
                assert sp_sb is not None
                return sp_sb[:, off : off + 1]

            row_i = [0]

            def rowtile(nm):
                row_i[0] += 1
                return sb.tile([1, S], F32, name=f"r{nm}{row_i[0]}", tag="row", bufs=6)

            ln_i = [0]

            def layer_norm(r_tiles, goff, boff):
                """r_tiles: 6 bf16 [128, S] tiles. Returns 6 new bf16 tiles.

                Critical-path layout: the -mean broadcast happens as soon as
                the plain sum is done, so the centering pass (pass 1) for
                every tile overlaps the sum-of-squares / rstd chain; only the
                final multiply by rstd (pass 2) waits for the full chain.
                Broadcasts are ones-matmuls on the PE (keeps HAM warm during
                the LN bubble)."""
                ln_i[0] += 1
                i = ln_i[0]
                s_ps = ps.tile([1, S], F32, name=f"lns{i}", tag="psrow", bufs=1)
                s2_ps = ps.tile([1, S], F32, name=f"lns2{i}", tag="pscore", bufs=3)
                for k in range(KT):
                    nc.tensor.matmul(
                        s_ps, ones1, r_tiles[k], start=(k == 0), stop=(k == KT - 1)
                    )
                # -mean row + its broadcast (early; feeds pass 1). The
                # broadcast is a K=1 fp32 ones-matmul into PSUM: exact, and it
                # is real PE activity in the middle of the LN bubble.
                mb_row = rowtile("mb")
                nc.vector.tensor_scalar_mul(mb_row, s_ps, -1.0 / H)
                mb_b = sb.tile([128, S], F32, name=f"lnmb{i}", tag="bc", bufs=4)
                nc.gpsimd.partition_broadcast(mb_b, mb_row)
                # sum of squares -> rstd
                for k in range(KT):
                    sq = sb.tile([128, S], BF16, name=f"lnsq{i}_{k}", tag="xsq", bufs=2)
                    nc.scalar.activation(sq, r_tiles[k], AF.Square)
                    nc.tensor.matmul(
                        s2_ps, ones1, sq, start=(k == 0), stop=(k == KT - 1)
                    )
                mm_row = rowtile("mm")  # mb^2 = (s/H)^2
                nc.vector.tensor_mul(mm_row, mb_row, mb_row)
                u_row = rowtile("u")  # u = s2 - H*mb^2  (var = u/H)
                nc.vector.scalar_tensor_tensor(
                    u_row, mm_row, -float(H), s2_ps, op0=OP.mult, op1=OP.add
                )
                std_row = rowtile("std")  # sqrt(u/H + eps)
                nc.scalar.activation(
                    std_row, u_row, AF.Sqrt, bias=eps_sc[:, :], scale=1.0 / H
                )
                rr_row = rowtile("rr")
                nc.vector.reciprocal_approx_fast(rr_row, std_row)
                rb_b = sb.tile([128, S], F32, name=f"lnrb{i}", tag="bc", bufs=4)
                nc.gpsimd.partition_broadcast(rb_b, rr_row)
                # dummy weight loads: PE activity pulses spaced through the LN
                # bubble (dep on successive row tiles of the rstd chain) so
                # the HAM monitor never demotes the clock. Values are unused —
                # every matmul issues its own LDWEIGHTS.
                for src in (u_row, std_row, rr_row):
                    v = src.bitcast(BF16)
                    for b in range(3):
                        nc.tensor.ldweights(v[:1, b * 128 : (b + 1) * 128])
                out = []
                for k in range(KT):
                    t1 = sb.tile([128, S], F32, name=f"lnt{i}_{k}", tag="lnt", bufs=6)
                    nc.vector.tensor_add(t1, r_tiles[k], mb_b)  # pass 1 (early)
                    o = sb.tile([128, S], BF16, name=f"lno{i}_{k}", tag="h", bufs=16)
                    if ln_triv:
                        nc.vector.tensor_mul(o, t1, rb_b)  # pass 2
                    else:
                        t2 = sb.tile(
                            [128, S], F32, name=f"lnu{i}_{k}", tag="lnt2", bufs=2
                        )
                        nc.vector.tensor_mul(t2, t1, rb_b)
                        nc.vector.tensor_scalar(
                            o, t2, pcol(goff + k), pcol(boff + k),
                            op0=OP.mult, op1=OP.add,
                        )
                    out.append(o)
                return out

            # ---- embeddings LN ----
            x0 = []
            for k in range(KT):
                xt = sb.tile([128, S], BF16, name=f"x0_{k}", tag="h", bufs=16)
                nc.sync.dma_start(xt, x0T[k * 128 : (k + 1) * 128, :])
                x0.append(xt)
            h = layer_norm(x0, 0, 6)
            act_warmup(99, AF.Exp)

            for l in range(NL):
                base = 12 + SPL * l
                wa1 = sb.tile([128, HALF], BF16, name=f"wa1_{l}", tag="w", bufs=4)
                nc.sync.dma_start(wa1, wA[l, :, 0:HALF])
                wa2 = sb.tile([128, HALF], BF16, name=f"wa2_{l}", tag="w", bufs=4)
                nc.sync.dma_start(wa2, wA[l, :, HALF:WAC])

                # V first (token-major with a ones column per head:
                # v_sb[t] is [128 tokens, 12 heads, 65], col 64 == 1.0)
                v_sb = []
                for t in range(TT):
                    vt = sb.tile([128, NH, DH + 1], BF16, name=f"v{l}_{t}", tag="v",
                                 bufs=5)
                    nc.vector.memset(vt[:, :, DH : DH + 1], 1.0)
                    v_sb.append(vt)
                for t in range(TT):
                    for n in range(2):
                        pv = ps.tile(
                            [128, 384], F32, name=f"pv{l}_{t}_{n}", tag="pp", bufs=2
                        )
                        for k in range(KT):
                            nc.tensor.matmul(
                                pv,
                                h[k][:, t * 128 : (t + 1) * 128],
                                wa2[:, k * H + n * 384 : k * H + (n + 1) * 384],
                                start=(k == 0),
                                stop=(k == KT - 1),
                            )
                        dstv = v_sb[t][:, n * 6 : (n + 1) * 6, 0:DH]
                        pv3 = pv.rearrange("p (a b) -> p a b", a=6)
                        if zv:
                            nc.vector.tensor_copy(dstv, pv3)
                        else:
                            assert srow_sb is not None
                            bvb = sb.tile(
                                [128, 384], F32, name=f"bvb{l}_{t}_{n}", tag="bvb",
                                bufs=2,
                            )
                            nc.gpsimd.partition_broadcast(
                                bvb, srow_sb[:, l * H + n * 384 : l * H + (n + 1) * 384]
                            )
                            nc.vector.tensor_add(
                                dstv, pv3, bvb.rearrange("p (a b) -> p a b", a=6)
                            )

                # Q^T/K^T projections software-pipelined INTO the attention
                # head loop: the projection matmuls for h-tile m+1 are emitted
                # between the scores and AV of head pair m, so the in-order PE
                # stream has dense work while ACT chases the 48 exp evictions
                # (attention alone is exp-bound on ACT).
                def qk_tile(m):
                    pair = []
                    for which, woff, bias_off, bz in (
                        (0, 0, base + 24, zq),
                        (1, 4608, base + 30, zk),
                    ):
                        pq = ps.tile(
                            [128, S], F32, name=f"pq{l}_{which}_{m}", tag="pp", bufs=2
                        )
                        for k in range(KT):
                            nc.tensor.matmul(
                                pq,
                                wa1[:, woff + k * H + m * 128 : woff + k * H + (m + 1) * 128],
                                h[k],
                                start=(k == 0),
                                stop=(k == KT - 1),
                            )
                        o = sb.tile(
                            [128, S], BF16, name=f"qk{l}_{which}_{m}", tag="qkt",
                            bufs=6,
                        )
                        if bz:
                            nc.vector.tensor_copy(o, pq)
                        else:
                            nc.vector.tensor_scalar_add(o, pq, pcol(bias_off + m))
                        pair.append(o)
                    return pair

                ctxt = [
                    sb.tile([128, S], BF16, name=f"ctx{l}_{k}", tag="ctx", bufs=8)
                    for k in range(KT)
                ]

                def head_scores(hd, qt_m, kt_m):
                    poff = (hd % 2) * DH
                    kt_sl = kt_m[poff : poff + DH, :]
                    qt_sl = qt_m[poff : poff + DH, :]
                    e_tiles = []
                    for t4 in range(TT):
                        pss = ps.tile(
                            [128, S], F32, name=f"psS{l}_{hd}_{t4}", tag="pscore",
                            bufs=3,
                        )
                        nc.tensor.matmul(
                            pss,
                            kt_sl[:, t4 * 128 : (t4 + 1) * 128],
                            qt_sl,
                            start=True,
                            stop=True,
                        )
                        et = sb.tile(
                            [128, S], BF16, name=f"e{l}_{hd}_{t4}", tag="e", bufs=10
                        )
                        nc.scalar.activation(et, pss, AF.Exp, scale=0.125)
                        e_tiles.append(et)
                    return e_tiles

                def head_av(hd, e_tiles):
                    kti = hd // 2
                    poff = (hd % 2) * DH
                    pc = ps.tile([DH + 1, S], F32, name=f"pc{l}_{hd}", tag="pctx",
                                 bufs=2)
                    for t4 in range(TT):
                        nc.tensor.matmul(
                            pc,
                            v_sb[t4][:, hd, :],
                            e_tiles[t4],
                            start=(t4 == 0),
                            stop=(t4 == TT - 1),
                        )
                    # free the PSUM bank fast: evict unnormalized ctx via ACT,
                    # then normalize off the PE/PSUM critical path on DVE.
                    # (reciprocal_approx_fast drops nonzero partition offsets,
                    # so stage Z to a partition-0 tile first)
                    zz = rowtile("zz")
                    nc.scalar.activation(zz, pc[DH : DH + 1, :], AF.Copy)
                    rz = rowtile("rz")
                    nc.vector.reciprocal_approx_fast(rz, zz)
                    rzb = sb.tile([DH, S], F32, name=f"rzb{l}_{hd}", tag="rzb",
                                  bufs=3)
                    nc.gpsimd.partition_broadcast(rzb, rz)
                    nc.vector.tensor_mul(ctxt[kti][poff : poff + DH, :], pc[0:DH, :],
                                         rzb)

                qk_cur = qk_tile(0)
                for m in range(KT):
                    e0 = head_scores(2 * m, qk_cur[0], qk_cur[1])
                    e1 = head_scores(2 * m + 1, qk_cur[0], qk_cur[1])
                    if m + 1 < KT:
                        qk_next = qk_tile(m + 1)  # PE filler while exps run
                    head_av(2 * m, e0)
                    head_av(2 * m + 1, e1)
                    if m + 1 < KT:
                        qk_cur = qk_next

                # Wo projection + residual
                r1 = []
                for m in range(KT):
                    po = ps.tile([128, S], F32, name=f"po{l}_{m}", tag="pp", bufs=2)
                    for k in range(KT):
                        nc.tensor.matmul(
                            po,
                            wa2[:, 4608 + k * H + m * 128 : 4608 + k * H + (m + 1) * 128],
                            ctxt[k],
                            start=(k == 0),
                            stop=(k == KT - 1),
                        )
                    o = sb.tile([128, S], BF16, name=f"r1_{l}_{m}", tag="h", bufs=16)
                    if zo:
                        nc.vector.tensor_add(o, h[m], po)
                    else:
                        nc.vector.scalar_tensor_tensor(
                            o, po, pcol(base + 36 + m), h[m], op0=OP.add, op1=OP.add
                        )
                    r1.append(o)
                h1 = layer_norm(r1, base + 0, base + 6)

                # FFN
                wb1 = sb.tile([128, HALF], BF16, name=f"wb1_{l}", tag="w", bufs=4)
                nc.sync.dma_start(wb1, wB[l, :, 0:HALF])
                wb2 = sb.tile([128, HALF], BF16, name=f"wb2_{l}", tag="w", bufs=4)
                nc.sync.dma_start(wb2, wB[l, :, HALF:WAC])
                g = []
                for fm in range(FKT):
                    pg = ps.tile([128, S], F32, name=f"pg{l}_{fm}", tag="pp", bufs=2)
                    for k in range(KT):
                        wh = wb1 if k < 3 else wb2
                        off = (k % 3) * FF + fm * 128
                        nc.tensor.matmul(
                            pg, wh[:, off : off + 128], h1[k],
                            start=(k == 0), stop=(k == KT - 1),
                        )
                    go = sb.tile([128, S], BF16, name=f"g{l}_{fm}", tag="g", bufs=24)
                    if zi:
                        nc.scalar.activation(go, pg, AF.Gelu)
                    else:
                        nc.scalar.activation(go, pg, AF.Gelu, bias=pcol(base + 48 + fm))
                    g.append(go)

                wc1 = sb.tile([128, HALF], BF16, name=f"wc1_{l}", tag="w", bufs=4)
                nc.sync.dma_start(wc1, wC[l, :, 0:HALF])
                wc2 = sb.tile([128, HALF], BF16, name=f"wc2_{l}", tag="w", bufs=4)
                nc.sync.dma_start(wc2, wC[l, :, HALF:WAC])
                r2 = []
                for m in range(KT):
                    pf = ps.tile([128, S], F32, name=f"pf{l}_{m}", tag="pp", bufs=2)
                    for k in range(FKT):
                        wh = wc1 if k < 12 else wc2
                        off = (k % 12) * H + m * 128
                        nc.tensor.matmul(
                            pf, wh[:, off : off + 128], g[k],
                            start=(k == 0), stop=(k == FKT - 1),
                        )
                    o = sb.tile([128, S], BF16, name=f"r2_{l}_{m}", tag="h", bufs=16)
                    if zo2:
                        nc.vector.tensor_add(o, h1[m], pf)
                    else:
                        nc.vector.scalar_tensor_tensor(
                            o, pf, pcol(base + 42 + m), h1[m], op0=OP.add, op1=OP.add
                        )
                    r2.append(o)
                h = layer_norm(r2, base + 12, base + 18)
                if l + 1 < NL:
                    # pull the Exp LUT load into the next layer's QKV phase so
                    # the first scores->exp of its attention doesn't stall
                    act_warmup(l, AF.Exp)

            # classifier head: logits[t, :] (cls_b added on host)
            for t in range(TT):
                pcl = ps.tile([128, S], F32, name=f"pcls{t}", tag="pp", bufs=2)
                for k in range(KT):
                    nc.tensor.matmul(
                        pcl[:, 0:NT],
                        h[k][:, t * 128 : (t + 1) * 128],
                        sb_cls[:, k * NT : (k + 1) * NT],
                        start=(k == 0),
                        stop=(k == KT - 1),
                    )
                ot = sb.tile([128, NT], F32, name=f"lg{t}", tag="lg", bufs=4)
                nc.scalar.activation(ot, pcl[:, 0:NT], AF.Copy)
                nc.sync.dma_start(logits[t * 128 : (t + 1) * 128, :], ot)

    nc.finalize()
    return nc


class _Runner:
    """Cached jitted SPMD executor (mirrors bass2jax.run_bass_via_pjrt)."""

    def __init__(self, nc, n_cores=8):
        import jax
        from jax.experimental.shard_map import shard_map
        from jax.sharding import Mesh, PartitionSpec
        from concourse import bass2jax, mybir as _mybir

        bass2jax.install_neuronx_cc_hook()
        self.n_cores = n_cores
        partition_name = (
            nc.partition_id_tensor.name if nc.partition_id_tensor else None
        )
        in_names, out_names, out_avals, zero_outs = [], [], [], []
        for alloc in nc.m.functions[0].allocations:
            if not isinstance(alloc, _mybir.MemoryLocationSet):
                continue
            name = alloc.memorylocations[0].name
            if alloc.kind == "ExternalInput":
                if name != partition_name:
                    in_names.append(name)
            elif alloc.kind == "ExternalOutput":
                shape = tuple(alloc.tensor_shape)
                dtype = _mybir.dt.np(alloc.dtype)
                out_names.append(name)
                out_avals.append(jax.core.ShapedArray(shape, dtype))
                zero_outs.append(np.zeros(shape, dtype))
        self.in_names = list(in_names)
        self.out_names = out_names
        self.out_avals = out_avals
        self.zero_outs = zero_outs
        n_params = len(in_names)
        n_outs = len(out_avals)
        donate = tuple(range(n_params, n_params + n_outs))
        all_in_names = tuple(
            in_names + out_names + ([partition_name] if partition_name else [])
        )

        def _body(*args):
            operands = list(args)
            if partition_name is not None:
                operands.append(bass2jax.partition_id_tensor())
            outs = bass2jax._bass_exec_p.bind(
                *operands,
                out_avals=tuple(out_avals),
                in_names=all_in_names,
                out_names=tuple(out_names),
                lowering_input_output_aliases=(),
                sim_require_finite=True,
                sim_require_nnan=True,
                nc=nc,
            )
            return tuple(outs)

        devices = jax.devices()[:n_cores]
        mesh = Mesh(np.asarray(devices), ("core",))
        in_specs = (PartitionSpec("core"),) * (n_params + n_outs)
        out_specs = (PartitionSpec("core"),) * n_outs
        self.sharded = jax.jit(
            shard_map(
                _body, mesh=mesh, in_specs=in_specs, out_specs=out_specs,
                check_rep=False,
            ),
            donate_argnums=donate,
            keep_unused=True,
        )

    def run(self, in_maps):
        nc_ = self.n_cores
        concat_in = [
            np.concatenate([np.asarray(m[name]) for m in in_maps], axis=0)
            for name in self.in_names
        ]
        concat_zeros = [
            np.zeros((nc_ * z.shape[0], *z.shape[1:]), z.dtype)
            for z in self.zero_outs
        ]
        out_arrs = self.sharded(*concat_in, *concat_zeros)
        return [
            {
                name: np.asarray(out_arrs[i]).reshape(
                    nc_, *self.out_avals[i].shape
                )[c]
                for i, name in enumerate(self.out_names)
            }
            for c in range(nc_)
        ]


_PACK_CACHE = {}
_PROG_CACHE = {}


def _pack_params(params):
    key = id(params.get("Wq", None))
    hit = _PACK_CACHE.get("k")
    if hit is not None and hit[0] == key:
        return hit[1], hit[2]
    p = {k: np.asarray(v) for k, v in params.items()}

    def as_blob(w, nk, cols):
        # [nk*128, cols] -> [128, nk*cols] with k-tile-major column order
        return np.ascontiguousarray(
            w.reshape(nk, 128, cols).transpose(1, 0, 2).reshape(128, nk * cols)
        ).astype(ml_dtypes.bfloat16)

    wa = np.stack(
        [
            np.concatenate(
                [as_blob(p[nm][l], KT, H) for nm in ("Wq", "Wk", "Wv", "Wo")], axis=1
            )
            for l in range(NL)
        ]
    )
    wb = np.stack([as_blob(p["Wi"][l], KT, FF) for l in range(NL)])
    wc = np.stack([as_blob(p["Wo2"][l], FKT, H) for l in range(NL)])

    def cols6(v):
        return v.reshape(-1, 128).T.astype(np.float32)  # [128, len/128]

    smallp = np.zeros((128, SPC), np.float32)
    smallp[:, 0:6] = cols6(p["emb_ln_g"])
    smallp[:, 6:12] = cols6(p["emb_ln_b"])
    for l in range(NL):
        b = 12 + SPL * l
        smallp[:, b : b + 6] = cols6(p["ln1_g"][l])
        smallp[:, b + 6 : b + 12] = cols6(p["ln1_b"][l])
        smallp[:, b + 12 : b + 18] = cols6(p["ln2_g"][l])
        smallp[:, b + 18 : b + 24] = cols6(p["ln2_b"][l])
        smallp[:, b + 24 : b + 30] = cols6(p["bq"][l])
        smallp[:, b + 30 : b + 36] = cols6(p["bk"][l])
        smallp[:, b + 36 : b + 42] = cols6(p["bo"][l])
        smallp[:, b + 42 : b + 48] = cols6(p["bo2"][l])
        smallp[:, b + 48 : b + 72] = cols6(p["bi"][l])
    smallb = (
        p["cls_W"].reshape(KT, 128, NT).transpose(1, 0, 2).reshape(128, KT * NT)
    ).astype(ml_dtypes.bfloat16)
    smallrow = p["bv"].reshape(1, NL * H).astype(np.float32)

    z = lambda a: bool(np.all(a == 0.0))
    flags = (
        z(p["bq"]), z(p["bk"]), z(p["bv"]), z(p["bo"]), z(p["bi"]), z(p["bo2"]),
        bool(
            np.all(p["ln1_g"] == 1) and np.all(p["ln2_g"] == 1)
            and np.all(p["emb_ln_g"] == 1) and z(p["ln1_b"]) and z(p["ln2_b"])
            and z(p["emb_ln_b"])
        ),
    )
    blobs = {
        "wA": wa, "wB": wb, "wC": wc,
        "smallp": smallp, "smallb": smallb, "smallrow": smallrow,
    }
    _PACK_CACHE["k"] = (key, blobs, flags)
    return blobs, flags


def _logsumexp(x, axis):
    m = np.max(x, axis=axis, keepdims=True)
    return np.squeeze(m, axis) + np.log(np.sum(np.exp(x - m), axis=axis))


def _crf_nll(params, logits, labels, mask):
    """torchcrf-style -sum(llh); mirrors the jax reference, in float64."""
    trans = np.asarray(params["crf_trans"], np.float64)
    cstart = np.asarray(params["crf_start"], np.float64)
    cend = np.asarray(params["crf_end"], np.float64)
    lg = logits.astype(np.float64)
    maskf = mask.astype(np.float64)
    Bb, Ss = labels.shape
    em_tag = np.take_along_axis(lg, labels[..., None], axis=2)[..., 0]
    trans_sc = trans[labels[:, :-1], labels[:, 1:]]
    num = cstart[labels[:, 0]] + em_tag[:, 0]
    num = num + np.sum(maskf[:, 1:] * (trans_sc + em_tag[:, 1:]), axis=1)
    seq_ends = mask.astype(np.int64).sum(1) - 1
    num = num + cend[labels[np.arange(Bb), seq_ends]]
    alpha = cstart[None] + lg[:, 0]
    for t in range(1, Ss):
        nxt = _logsumexp(alpha[:, :, None] + trans[None] + lg[:, t][:, None, :], 1)
        alpha = np.where(maskf[:, t][:, None] > 0, nxt, alpha)
    den = _logsumexp(alpha + cend[None], 1)
    return np.sum(den - num)


def _reference_numpy(input_ids, attention_mask, token_type_ids, labels, params):
    """Full-precision numpy fallback (used only if attention_mask has zeros,
    which the device kernel does not model in the attention bias)."""
    p = {k: np.asarray(v, np.float64) if np.asarray(v).dtype.kind == "f" else
         np.asarray(v) for k, v in params.items()}

    def ln(x, g, b):
        m = x.mean(-1, keepdims=True)
        v_ = ((x - m) ** 2).mean(-1, keepdims=True)
        return (x - m) / np.sqrt(v_ + EPS) * g + b

    x = p["word_emb"][input_ids] + p["pos_emb"][None, :S] + p["type_emb"][token_type_ids]
    x = ln(x, p["emb_ln_g"], p["emb_ln_b"])
    bias = np.where(attention_mask[:, None, None, :] > 0, 0.0, -1e9)
    from scipy.special import erf  # available in the image; exact gelu

    for l in range(NL):
        q = (x @ p["Wq"][l] + p["bq"][l]).reshape(B, S, NH, DH)
        k = (x @ p["Wk"][l] + p["bk"][l]).reshape(B, S, NH, DH)
        v_ = (x @ p["Wv"][l] + p["bv"][l]).reshape(B, S, NH, DH)
        sc = np.einsum("bqhd,bkhd->bhqk", q, k) / np.sqrt(DH) + bias
        sc = sc - sc.max(-1, keepdims=True)
        a = np.exp(sc)
        a /= a.sum(-1, keepdims=True)
        ctx = np.einsum("bhqk,bkhd->bqhd", a, v_).reshape(B, S, H)
        x = ln(x + ctx @ p["Wo"][l] + p["bo"][l], p["ln1_g"][l], p["ln1_b"][l])
        f = x @ p["Wi"][l] + p["bi"][l]
        f = 0.5 * f * (1.0 + erf(f / np.sqrt(2.0)))
        x = ln(x + f @ p["Wo2"][l] + p["bo2"][l], p["ln2_g"][l], p["ln2_b"][l])
    return (x @ p["cls_W"] + p["cls_b"]).astype(np.float32)


def _get_runner(flags):
    hit = _PROG_CACHE.get(flags)
    if hit is None:
        nc = _build_program(flags)
        hit = _Runner(nc, 8)
        _PROG_CACHE[flags] = hit
    return hit


def kernel(input_ids, attention_mask, token_type_ids, labels, params):
    ids = np.asarray(input_ids).astype(np.int64)
    tt = np.asarray(token_type_ids).astype(np.int64)
    mask = np.asarray(attention_mask)
    lab = np.asarray(labels).astype(np.int64)

    if not np.all(np.asarray(mask) > 0):
        lg = _reference_numpy(ids, np.asarray(mask), tt, lab, params)
        return np.asarray(_crf_nll(params, lg, lab, mask), np.float32)

    blobs, flags = _pack_params(params)
    runner = _get_runner(flags)

    we = np.asarray(params["word_emb"], np.float32)
    pe = np.asarray(params["pos_emb"], np.float32)
    te = np.asarray(params["type_emb"], np.float32)
    x0 = we[ids] + pe[None, :S] + te[tt]  # [B, S, H] fp32
    x0T = np.ascontiguousarray(x0.transpose(0, 2, 1)).astype(ml_dtypes.bfloat16)

    in_maps = []
    for c in range(B):
        m = {"x0T": x0T[c]}
        for name in runner.in_names:
            if name != "x0T":
                m[name] = blobs[name]
        in_maps.append(m)
    res = runner.run(in_maps)
    logits = np.stack([res[c]["logits"] for c in range(B)]).astype(np.float32)
    logits = logits + np.asarray(params["cls_b"], np.float32)[None, None, :]
    return np.asarray(_crf_nll(params, logits, lab, mask), np.float32)


# revision 35
# speedup vs baseline: 1.1480x; 1.0233x over previous
"""BERT-base + CRF loss kernel for 8 Trainium2 NeuronCores.

Strategy (data-parallel over batch, B=8 -> 1 sequence per core):
  - Host: embedding gather (+pos/+type), pack weights into bf16 DMA blobs.
  - Device (per core): 12-layer BERT encoder + classifier head on one
    [512, 768] sequence, activations kept in transposed [H, T] layout
    (hidden on partitions) so no transposes are ever needed. Attention
    computes exp(S^T) unnormalized; a ones-column baked into V makes the
    softmax denominator fall out of the AV matmul; normalization is fused
    into the ctx eviction. Matmuls in bf16 (1 col/cycle), fp32 PSUM.
  - Host: CRF negative log-likelihood on the [B, 512, 9] emissions
    (tiny sequential scan, bad fit for the PE) and the final scalar sum.
"""

import os
import sys

for _p in ("/opt/trn_rl_repo", "/root/.axon_site/_ro/trn_rl_repo"):
    if os.path.isdir(_p) and _p not in sys.path:
        sys.path.insert(0, _p)

import numpy as np
import ml_dtypes

import concourse.bass as bass
import concourse.tile as tile
from concourse import bacc, mybir

BF16 = mybir.dt.bfloat16
F32 = mybir.dt.float32
AF = mybir.ActivationFunctionType
OP = mybir.AluOpType

V, H, NL, NH, DH, FF, MAXPOS, NT = 30522, 768, 12, 12, 64, 3072, 512, 9
B, S = 8, 512
EPS = 1e-12
KT = H // 128        # 6 hidden k-tiles
FKT = FF // 128      # 24 ffn k-tiles
TT = S // 128        # 4 token tiles
WAC = 18432          # blob cols per layer
HALF = WAC // 2      # 9216
SPL = 72             # smallp cols per layer
SPC = 12 + SPL * NL  # smallp total cols


def _build_program(flags):
    """flags = (zq, zk, zv, zo, zi, zo2, ln_triv); z* True means that bias is
    all-zero (skip adding it), ln_triv True means every LN gamma==1, beta==0."""
    zq, zk, zv, zo, zi, zo2, ln_triv = flags
    nc = bacc.Bacc("TRN2", target_bir_lowering=False, debug=False)

    x0T = nc.dram_tensor("x0T", [H, S], BF16, kind="ExternalInput")
    wA = nc.dram_tensor("wA", [NL, 128, WAC], BF16, kind="ExternalInput")
    wB = nc.dram_tensor("wB", [NL, 128, WAC], BF16, kind="ExternalInput")
    wC = nc.dram_tensor("wC", [NL, 128, WAC], BF16, kind="ExternalInput")
    need_smallp = (not ln_triv) or not (zq and zk and zo and zi and zo2)
    smallp = (
        nc.dram_tensor("smallp", [128, SPC], F32, kind="ExternalInput")
        if need_smallp
        else None
    )
    smallb = nc.dram_tensor("smallb", [128, KT * NT], BF16, kind="ExternalInput")
    smallrow = (
        nc.dram_tensor("smallrow", [1, NL * H], F32, kind="ExternalInput")
        if not zv
        else None
    )
    logits = nc.dram_tensor("logits", [S, NT], F32, kind="ExternalOutput")

    with tile.TileContext(nc) as tc:
        with (
            tc.tile_pool(name="sb", bufs=1) as sb,
            tc.tile_pool(name="ps", bufs=1, space="PSUM") as ps,
        ):
            ones1 = sb.tile([128, 1], BF16, name="ones1")
            nc.vector.memset(ones1, 1.0)
            onesr = sb.tile([1, 128], F32, name="onesr")
            nc.vector.memset(onesr, 1.0)
            eps_sc = sb.tile([1, 1], F32, name="eps_sc")
            nc.vector.memset(eps_sc, EPS)
            warm_sc = sb.tile([1, 1], F32, name="warm_sc")
            nc.vector.memset(warm_sc, 1.0)

            def act_warmup(i, func):
                # touch an ACT LUT while the PE is busy elsewhere so the
                # 1.3us table load doesn't land on a critical path later
                w1 = sb.tile([1, 1], F32, name=f"aw{i}", tag="warm", bufs=2)
                nc.scalar.activation(w1, warm_sc, func)
            sp_sb = None
            if smallp is not None:
                sp_sb = sb.tile([128, SPC], F32, name="sp_sb")
                nc.sync.dma_start(sp_sb, smallp[:, :])
            sb_cls = sb.tile([128, KT * NT], BF16, name="sb_cls")
            nc.sync.dma_start(sb_cls, smallb[:, :])
            srow_sb = None
            if smallrow is not None:
                srow_sb = sb.tile([1, NL * H], F32, name="srow_sb")
                nc.sync.dma_start(srow_sb, smallrow[:, :])

            def pcol(off):
                assert sp_sb is not None
                return sp_sb[:, off : off + 1]

            row_i = [0]

            def rowtile(nm):
                row_i[0] += 1
                return sb.tile([1, S], F32, name=f"r{nm}{row_i[0]}", tag="row", bufs=6)

            ln_i = [0]

            def layer_norm(r_tiles, goff, boff):
                """r_tiles: 6 bf16 [128, S] tiles. Returns 6 new bf16 tiles.

                Critical-path layout: the -mean broadcast happens as soon as
                the plain sum is done, so the centering pass (pass 1) for
                every tile overlaps the sum-of-squares / rstd chain; only the
                final multiply by rstd (pass 2) waits for the full chain.
                Broadcasts are ones-matmuls on the PE (keeps HAM warm during
                the LN bubble)."""
                ln_i[0] += 1
                i = ln_i[0]
                s_ps = ps.tile([1, S], F32, name=f"lns{i}", tag="psrow", bufs=1)
                s2_ps = ps.tile([1, S], F32, name=f"lns2{i}", tag="pscore", bufs=3)
                for k in range(KT):
                    nc.tensor.matmul(
                        s_ps, ones1, r_tiles[k], start=(k == 0), stop=(k == KT - 1)
                    )
                # -mean row + its broadcast (early; feeds pass 1). The
                # broadcast is a K=1 fp32 ones-matmul into PSUM: exact, and it
                # is real PE activity in the middle of the LN bubble.
                mb_row = rowtile("mb")
                nc.vector.tensor_scalar_mul(mb_row, s_ps, -1.0 / H)
                mb_b = sb.tile([128, S], F32, name=f"lnmb{i}", tag="bc", bufs=4)
                nc.gpsimd.partition_broadcast(mb_b, mb_row)
                # sum of squares -> rstd
                for k in range(KT):
                    sq = sb.tile([128, S], BF16, name=f"lnsq{i}_{k}", tag="xsq", bufs=2)
                    nc.scalar.activation(sq, r_tiles[k], AF.Square)
                    nc.tensor.matmul(
                        s2_ps, ones1, sq, start=(k == 0), stop=(k == KT - 1)
                    )
                mm_row = rowtile("mm")  # mb^2 = (s/H)^2
                nc.vector.tensor_mul(mm_row, mb_row, mb_row)
                u_row = rowtile("u")  # u = s2 - H*mb^2  (var = u/H)
                nc.vector.scalar_tensor_tensor(
                    u_row, mm_row, -float(H), s2_ps, op0=OP.mult, op1=OP.add
                )
                std_row = rowtile("std")  # sqrt(u/H + eps)
                nc.scalar.activation(
                    std_row, u_row, AF.Sqrt, bias=eps_sc[:, :], scale=1.0 / H
                )
                rr_row = rowtile("rr")
                nc.vector.reciprocal_approx_fast(rr_row, std_row)
                rb_b = sb.tile([128, S], F32, name=f"lnrb{i}", tag="bc", bufs=4)
                nc.gpsimd.partition_broadcast(rb_b, rr_row)
                # dummy weight loads: PE activity pulses spaced through the LN
                # bubble (dep on successive row tiles of the rstd chain) so
                # the HAM monitor never demotes the clock. Values are unused —
                # every matmul issues its own LDWEIGHTS.
                for src in (u_row, std_row, rr_row):
                    v = src.bitcast(BF16)
                    for b in range(3):
                        nc.tensor.ldweights(v[:1, b * 128 : (b + 1) * 128])
                out = []
                # pass1/pass2 interleaved per tile AFTER the rstd chain in
                # program order, so the DVE runs the chain that gates FFN
                # start first and h1[0] lands ~2 ops after rb_b
                for k in range(KT):
                    t1 = sb.tile([128, S], F32, name=f"lnt{i}_{k}", tag="lnt", bufs=6)
                    nc.vector.tensor_add(t1, r_tiles[k], mb_b)  # pass 1
                    o = sb.tile([128, S], BF16, name=f"lno{i}_{k}", tag="h", bufs=16)
                    if ln_triv:
                        nc.vector.tensor_mul(o, t1, rb_b)  # pass 2
                    else:
                        t2 = sb.tile(
                            [128, S], F32, name=f"lnu{i}_{k}", tag="lnt2", bufs=2
                        )
                        nc.vector.tensor_mul(t2, t1, rb_b)
                        nc.vector.tensor_scalar(
                            o, t2, pcol(goff + k), pcol(boff + k),
                            op0=OP.mult, op1=OP.add,
                        )
                    out.append(o)
                return out

            # ---- embeddings LN ----
            x0 = []
            for k in range(KT):
                xt = sb.tile([128, S], BF16, name=f"x0_{k}", tag="h", bufs=16)
                nc.sync.dma_start(xt, x0T[k * 128 : (k + 1) * 128, :])
                x0.append(xt)
            h = layer_norm(x0, 0, 6)
            act_warmup(99, AF.Exp)

            for l in range(NL):
                base = 12 + SPL * l
                wa1 = sb.tile([128, HALF], BF16, name=f"wa1_{l}", tag="w", bufs=4)
                nc.sync.dma_start(wa1, wA[l, :, 0:HALF])
                wa2 = sb.tile([128, HALF], BF16, name=f"wa2_{l}", tag="w", bufs=4)
                nc.sync.dma_start(wa2, wA[l, :, HALF:WAC])

                # V first (token-major with a ones column per head:
                # v_sb[t] is [128 tokens, 12 heads, 65], col 64 == 1.0)
                v_sb = []
                for t in range(TT):
                    vt = sb.tile([128, NH, DH + 1], BF16, name=f"v{l}_{t}", tag="v",
                                 bufs=5)
                    nc.vector.memset(vt[:, :, DH : DH + 1], 1.0)
                    v_sb.append(vt)
                for t in range(TT):
                    for n in range(2):
                        pv = ps.tile(
                            [128, 384], F32, name=f"pv{l}_{t}_{n}", tag="pp", bufs=2
                        )
                        for k in range(KT):
                            nc.tensor.matmul(
                                pv,
                                h[k][:, t * 128 : (t + 1) * 128],
                                wa2[:, k * H + n * 384 : k * H + (n + 1) * 384],
                                start=(k == 0),
                                stop=(k == KT - 1),
                            )
                        dstv = v_sb[t][:, n * 6 : (n + 1) * 6, 0:DH]
                        pv3 = pv.rearrange("p (a b) -> p a b", a=6)
                        if zv:
                            nc.vector.tensor_copy(dstv, pv3)
                        else:
                            assert srow_sb is not None
                            bvb = sb.tile(
                                [128, 384], F32, name=f"bvb{l}_{t}_{n}", tag="bvb",
                                bufs=2,
                            )
                            nc.gpsimd.partition_broadcast(
                                bvb, srow_sb[:, l * H + n * 384 : l * H + (n + 1) * 384]
                            )
                            nc.vector.tensor_add(
                                dstv, pv3, bvb.rearrange("p (a b) -> p a b", a=6)
                            )

                # Q^T/K^T projections software-pipelined INTO the attention
                # head loop: the projection matmuls for h-tile m+1 are emitted
                # between the scores and AV of head pair m, so the in-order PE
                # stream has dense work while ACT chases the 48 exp evictions
                # (attention alone is exp-bound on ACT).
                def qk_tile(m):
                    pair = []
                    for which, woff, bias_off, bz in (
                        (0, 0, base + 24, zq),
                        (1, 4608, base + 30, zk),
                    ):
                        pq = ps.tile(
                            [128, S], F32, name=f"pq{l}_{which}_{m}", tag="pp", bufs=2
                        )
                        for k in range(KT):
                            nc.tensor.matmul(
                                pq,
                                wa1[:, woff + k * H + m * 128 : woff + k * H + (m + 1) * 128],
                                h[k],
                                start=(k == 0),
                                stop=(k == KT - 1),
                            )
                        o = sb.tile(
                            [128, S], BF16, name=f"qk{l}_{which}_{m}", tag="qkt",
                            bufs=6,
                        )
                        if bz:
                            nc.vector.tensor_copy(o, pq)
                        else:
                            nc.vector.tensor_scalar_add(o, pq, pcol(bias_off + m))
                        pair.append(o)
                    return pair

                ctxt = [
                    sb.tile([128, S], BF16, name=f"ctx{l}_{k}", tag="ctx", bufs=8)
                    for k in range(KT)
                ]

                def head_scores(hd, qt_m, kt_m):
                    poff = (hd % 2) * DH
                    kt_sl = kt_m[poff : poff + DH, :]
                    qt_sl = qt_m[poff : poff + DH, :]
                    e_tiles = []
                    for t4 in range(TT):
                        pss = ps.tile(
                            [128, S], F32, name=f"psS{l}_{hd}_{t4}", tag="pscore",
                            bufs=3,
                        )
                        nc.tensor.matmul(
                            pss,
                            kt_sl[:, t4 * 128 : (t4 + 1) * 128],
                            qt_sl,
                            start=True,
                            stop=True,
                        )
                        et = sb.tile(
                            [128, S], BF16, name=f"e{l}_{hd}_{t4}", tag="e", bufs=10
                        )
                        nc.scalar.activation(et, pss, AF.Exp, scale=0.125)
                        e_tiles.append(et)
                    return e_tiles

                def head_av(hd, e_tiles):
                    kti = hd // 2
                    poff = (hd % 2) * DH
                    pc = ps.tile([DH + 1, S], F32, name=f"pc{l}_{hd}", tag="pctx",
                                 bufs=2)
                    for t4 in range(TT):
                        nc.tensor.matmul(
                            pc,
                            v_sb[t4][:, hd, :],
                            e_tiles[t4],
                            start=(t4 == 0),
                            stop=(t4 == TT - 1),
                        )
                    # free the PSUM bank fast: evict unnormalized ctx via ACT,
                    # then normalize off the PE/PSUM critical path on DVE.
                    # (reciprocal_approx_fast drops nonzero partition offsets,
                    # so stage Z to a partition-0 tile first)
                    zz = rowtile("zz")
                    nc.vector.tensor_copy(zz, pc[DH : DH + 1, :])
                    rz = rowtile("rz")
                    nc.vector.reciprocal_approx_fast(rz, zz)
                    rzb = sb.tile([DH, S], F32, name=f"rzb{l}_{hd}", tag="rzb",
                                  bufs=3)
                    nc.gpsimd.partition_broadcast(rzb, rz)
                    nc.vector.tensor_mul(ctxt[kti][poff : poff + DH, :], pc[0:DH, :],
                                         rzb)

                qk_cur = qk_tile(0)
                for m in range(KT):
                    e0 = head_scores(2 * m, qk_cur[0], qk_cur[1])
                    e1 = head_scores(2 * m + 1, qk_cur[0], qk_cur[1])
                    if m + 1 < KT:
                        qk_next = qk_tile(m + 1)  # PE filler while exps run
                    head_av(2 * m, e0)
                    head_av(2 * m + 1, e1)
                    if m + 1 < KT:
                        qk_cur = qk_next

                # Wo projection + residual
                r1 = []
                for m in range(KT):
                    po = ps.tile([128, S], F32, name=f"po{l}_{m}", tag="pp", bufs=2)
                    for k in range(KT):
                        nc.tensor.matmul(
                            po,
                            wa2[:, 4608 + k * H + m * 128 : 4608 + k * H + (m + 1) * 128],
                            ctxt[k],
                            start=(k == 0),
                            stop=(k == KT - 1),
                        )
                    o = sb.tile([128, S], BF16, name=f"r1_{l}_{m}", tag="h", bufs=16)
                    if zo:
                        nc.vector.tensor_add(o, h[m], po)
                    else:
                        nc.vector.scalar_tensor_tensor(
                            o, po, pcol(base + 36 + m), h[m], op0=OP.add, op1=OP.add
                        )
                    r1.append(o)
                h1 = layer_norm(r1, base + 0, base + 6)

                # FFN
                wb1 = sb.tile([128, HALF], BF16, name=f"wb1_{l}", tag="w", bufs=4)
                nc.sync.dma_start(wb1, wB[l, :, 0:HALF])
                wb2 = sb.tile([128, HALF], BF16, name=f"wb2_{l}", tag="w", bufs=4)
                nc.sync.dma_start(wb2, wB[l, :, HALF:WAC])
                g = []
                for fm in range(FKT):
                    pg = ps.tile([128, S], F32, name=f"pg{l}_{fm}", tag="pp", bufs=2)
                    for k in range(KT):
                        wh = wb1 if k < 3 else wb2
                        off = (k % 3) * FF + fm * 128
                        nc.tensor.matmul(
                            pg, wh[:, off : off + 128], h1[k],
                            start=(k == 0), stop=(k == KT - 1),
                        )
                    go = sb.tile([128, S], BF16, name=f"g{l}_{fm}", tag="g", bufs=24)
                    if zi:
                        nc.scalar.activation(go, pg, AF.Gelu)
                    else:
                        nc.scalar.activation(go, pg, AF.Gelu, bias=pcol(base + 48 + fm))
                    g.append(go)

                wc1 = sb.tile([128, HALF], BF16, name=f"wc1_{l}", tag="w", bufs=4)
                nc.sync.dma_start(wc1, wC[l, :, 0:HALF])
                wc2 = sb.tile([128, HALF], BF16, name=f"wc2_{l}", tag="w", bufs=4)
                nc.sync.dma_start(wc2, wC[l, :, HALF:WAC])
                r2 = []
                for m in range(KT):
                    pf = ps.tile([128, S], F32, name=f"pf{l}_{m}", tag="pp", bufs=2)
                    for k in range(FKT):
                        wh = wc1 if k < 12 else wc2
                        off = (k % 12) * H + m * 128
                        nc.tensor.matmul(
                            pf, wh[:, off : off + 128], g[k],
                            start=(k == 0), stop=(k == FKT - 1),
                        )
                    o = sb.tile([128, S], BF16, name=f"r2_{l}_{m}", tag="h", bufs=16)
                    if zo2:
                        nc.vector.tensor_add(o, h1[m], pf)
                    else:
                        nc.vector.scalar_tensor_tensor(
                            o, pf, pcol(base + 42 + m), h1[m], op0=OP.add, op1=OP.add
                        )
                    r2.append(o)
                h = layer_norm(r2, base + 12, base + 18)
                if l + 1 < NL:
                    # pull the Exp LUT load into the next layer's QKV phase so
                    # the first scores->exp of its attention doesn't stall
                    act_warmup(l, AF.Exp)

            # classifier head: logits[t, :] (cls_b added on host)
            for t in range(TT):
                pcl = ps.tile([128, S], F32, name=f"pcls{t}", tag="pp", bufs=2)
                for k in range(KT):
                    nc.tensor.matmul(
                        pcl[:, 0:NT],
                        h[k][:, t * 128 : (t + 1) * 128],
                        sb_cls[:, k * NT : (k + 1) * NT],
                        start=(k == 0),
                        stop=(k == KT - 1),
                    )
                ot = sb.tile([128, NT], F32, name=f"lg{t}", tag="lg", bufs=4)
                nc.scalar.activation(ot, pcl[:, 0:NT], AF.Copy)
                nc.sync.dma_start(logits[t * 128 : (t + 1) * 128, :], ot)

    nc.finalize()
    return nc


class _Runner:
    """Cached jitted SPMD executor (mirrors bass2jax.run_bass_via_pjrt)."""

    def __init__(self, nc, n_cores=8):
        import jax
        from jax.experimental.shard_map import shard_map
        from jax.sharding import Mesh, PartitionSpec
        from concourse import bass2jax, mybir as _mybir

        bass2jax.install_neuronx_cc_hook()
        self.n_cores = n_cores
        partition_name = (
            nc.partition_id_tensor.name if nc.partition_id_tensor else None
        )
        in_names, out_names, out_avals, zero_outs = [], [], [], []
        for alloc in nc.m.functions[0].allocations:
            if not isinstance(alloc, _mybir.MemoryLocationSet):
                continue
            name = alloc.memorylocations[0].name
            if alloc.kind == "ExternalInput":
                if name != partition_name:
                    in_names.append(name)
            elif alloc.kind == "ExternalOutput":
                shape = tuple(alloc.tensor_shape)
                dtype = _mybir.dt.np(alloc.dtype)
                out_names.append(name)
                out_avals.append(jax.core.ShapedArray(shape, dtype))
                zero_outs.append(np.zeros(shape, dtype))
        self.in_names = list(in_names)
        self.out_names = out_names
        self.out_avals = out_avals
        self.zero_outs = zero_outs
        n_params = len(in_names)
        n_outs = len(out_avals)
        donate = tuple(range(n_params, n_params + n_outs))
        all_in_names = tuple(
            in_names + out_names + ([partition_name] if partition_name else [])
        )

        def _body(*args):
            operands = list(args)
            if partition_name is not None:
                operands.append(bass2jax.partition_id_tensor())
            outs = bass2jax._bass_exec_p.bind(
                *operands,
                out_avals=tuple(out_avals),
                in_names=all_in_names,
                out_names=tuple(out_names),
                lowering_input_output_aliases=(),
                sim_require_finite=True,
                sim_require_nnan=True,
                nc=nc,
            )
            return tuple(outs)

        devices = jax.devices()[:n_cores]
        mesh = Mesh(np.asarray(devices), ("core",))
        in_specs = (PartitionSpec("core"),) * (n_params + n_outs)
        out_specs = (PartitionSpec("core"),) * n_outs
        self.sharded = jax.jit(
            shard_map(
                _body, mesh=mesh, in_specs=in_specs, out_specs=out_specs,
                check_rep=False,
            ),
            donate_argnums=donate,
            keep_unused=True,
        )

    def run(self, in_maps):
        nc_ = self.n_cores
        concat_in = [
            np.concatenate([np.asarray(m[name]) for m in in_maps], axis=0)
            for name in self.in_names
        ]
        concat_zeros = [
            np.zeros((nc_ * z.shape[0], *z.shape[1:]), z.dtype)
            for z in self.zero_outs
        ]
        out_arrs = self.sharded(*concat_in, *concat_zeros)
        return [
            {
                name: np.asarray(out_arrs[i]).reshape(
                    nc_, *self.out_avals[i].shape
                )[c]
                for i, name in enumerate(self.out_names)
            }
            for c in range(nc_)
        ]


_PACK_CACHE = {}
_PROG_CACHE = {}


def _pack_params(params):
    key = id(params.get("Wq", None))
    hit = _PACK_CACHE.get("k")
    if hit is not None and hit[0] == key:
        return hit[1], hit[2]
    p = {k: np.asarray(v) for k, v in params.items()}

    def as_blob(w, nk, cols):
        # [nk*128, cols] -> [128, nk*cols] with k-tile-major column order
        return np.ascontiguousarray(
            w.reshape(nk, 128, cols).transpose(1, 0, 2).reshape(128, nk * cols)
        ).astype(ml_dtypes.bfloat16)

    wa = np.stack(
        [
            np.concatenate(
                [as_blob(p[nm][l], KT, H) for nm in ("Wq", "Wk", "Wv", "Wo")], axis=1
            )
            for l in range(NL)
        ]
    )
    wb = np.stack([as_blob(p["Wi"][l], KT, FF) for l in range(NL)])
    wc = np.stack([as_blob(p["Wo2"][l], FKT, H) for l in range(NL)])

    def cols6(v):
        return v.reshape(-1, 128).T.astype(np.float32)  # [128, len/128]

    smallp = np.zeros((128, SPC), np.float32)
    smallp[:, 0:6] = cols6(p["emb_ln_g"])
    smallp[:, 6:12] = cols6(p["emb_ln_b"])
    for l in range(NL):
        b = 12 + SPL * l
        smallp[:, b : b + 6] = cols6(p["ln1_g"][l])
        smallp[:, b + 6 : b + 12] = cols6(p["ln1_b"][l])
        smallp[:, b + 12 : b + 18] = cols6(p["ln2_g"][l])
        smallp[:, b + 18 : b + 24] = cols6(p["ln2_b"][l])
        smallp[:, b + 24 : b + 30] = cols6(p["bq"][l])
        smallp[:, b + 30 : b + 36] = cols6(p["bk"][l])
        smallp[:, b + 36 : b + 42] = cols6(p["bo"][l])
        smallp[:, b + 42 : b + 48] = cols6(p["bo2"][l])
        smallp[:, b + 48 : b + 72] = cols6(p["bi"][l])
    smallb = (
        p["cls_W"].reshape(KT, 128, NT).transpose(1, 0, 2).reshape(128, KT * NT)
    ).astype(ml_dtypes.bfloat16)
    smallrow = p["bv"].reshape(1, NL * H).astype(np.float32)

    z = lambda a: bool(np.all(a == 0.0))
    flags = (
        z(p["bq"]), z(p["bk"]), z(p["bv"]), z(p["bo"]), z(p["bi"]), z(p["bo2"]),
        bool(
            np.all(p["ln1_g"] == 1) and np.all(p["ln2_g"] == 1)
            and np.all(p["emb_ln_g"] == 1) and z(p["ln1_b"]) and z(p["ln2_b"])
            and z(p["emb_ln_b"])
        ),
    )
    blobs = {
        "wA": wa, "wB": wb, "wC": wc,
        "smallp": smallp, "smallb": smallb, "smallrow": smallrow,
    }
    _PACK_CACHE["k"] = (key, blobs, flags)
    return blobs, flags


def _logsumexp(x, axis):
    m = np.max(x, axis=axis, keepdims=True)
    return np.squeeze(m, axis) + np.log(np.sum(np.exp(x - m), axis=axis))


def _crf_nll(params, logits, labels, mask):
    """torchcrf-style -sum(llh); mirrors the jax reference, in float64."""
    trans = np.asarray(params["crf_trans"], np.float64)
    cstart = np.asarray(params["crf_start"], np.float64)
    cend = np.asarray(params["crf_end"], np.float64)
    lg = logits.astype(np.float64)
    maskf = mask.astype(np.float64)
    Bb, Ss = labels.shape
    em_tag = np.take_along_axis(lg, labels[..., None], axis=2)[..., 0]
    trans_sc = trans[labels[:, :-1], labels[:, 1:]]
    num = cstart[labels[:, 0]] + em_tag[:, 0]
    num = num + np.sum(maskf[:, 1:] * (trans_sc + em_tag[:, 1:]), axis=1)
    seq_ends = mask.astype(np.int64).sum(1) - 1
    num = num + cend[labels[np.arange(Bb), seq_ends]]
    alpha = cstart[None] + lg[:, 0]
    for t in range(1, Ss):
        nxt = _logsumexp(alpha[:, :, None] + trans[None] + lg[:, t][:, None, :], 1)
        alpha = np.where(maskf[:, t][:, None] > 0, nxt, alpha)
    den = _logsumexp(alpha + cend[None], 1)
    return np.sum(den - num)


def _reference_numpy(input_ids, attention_mask, token_type_ids, labels, params):
    """Full-precision numpy fallback (used only if attention_mask has zeros,
    which the device kernel does not model in the attention bias)."""
    p = {k: np.asarray(v, np.float64) if np.asarray(v).dtype.kind == "f" else
         np.asarray(v) for k, v in params.items()}

    def ln(x, g, b):
        m = x.mean(-1, keepdims=True)
        v_ = ((x - m) ** 2).mean(-1, keepdims=True)
        return (x - m) / np.sqrt(v_ + EPS) * g + b

    x = p["word_emb"][input_ids] + p["pos_emb"][None, :S] + p["type_emb"][token_type_ids]
    x = ln(x, p["emb_ln_g"], p["emb_ln_b"])
    bias = np.where(attention_mask[:, None, None, :] > 0, 0.0, -1e9)
    from scipy.special import erf  # available in the image; exact gelu

    for l in range(NL):
        q = (x @ p["Wq"][l] + p["bq"][l]).reshape(B, S, NH, DH)
        k = (x @ p["Wk"][l] + p["bk"][l]).reshape(B, S, NH, DH)
        v_ = (x @ p["Wv"][l] + p["bv"][l]).reshape(B, S, NH, DH)
        sc = np.einsum("bqhd,bkhd->bhqk", q, k) / np.sqrt(DH) + bias
        sc = sc - sc.max(-1, keepdims=True)
        a = np.exp(sc)
        a /= a.sum(-1, keepdims=True)
        ctx = np.einsum("bhqk,bkhd->bqhd", a, v_).reshape(B, S, H)
        x = ln(x + ctx @ p["Wo"][l] + p["bo"][l], p["ln1_g"][l], p["ln1_b"][l])
        f = x @ p["Wi"][l] + p["bi"][l]
        f = 0.5 * f * (1.0 + erf(f / np.sqrt(2.0)))
        x = ln(x + f @ p["Wo2"][l] + p["bo2"][l], p["ln2_g"][l], p["ln2_b"][l])
    return (x @ p["cls_W"] + p["cls_b"]).astype(np.float32)


def _get_runner(flags):
    hit = _PROG_CACHE.get(flags)
    if hit is None:
        nc = _build_program(flags)
        hit = _Runner(nc, 8)
        _PROG_CACHE[flags] = hit
    return hit


def kernel(input_ids, attention_mask, token_type_ids, labels, params):
    ids = np.asarray(input_ids).astype(np.int64)
    tt = np.asarray(token_type_ids).astype(np.int64)
    mask = np.asarray(attention_mask)
    lab = np.asarray(labels).astype(np.int64)

    if not np.all(np.asarray(mask) > 0):
        lg = _reference_numpy(ids, np.asarray(mask), tt, lab, params)
        return np.asarray(_crf_nll(params, lg, lab, mask), np.float32)

    blobs, flags = _pack_params(params)
    runner = _get_runner(flags)

    we = np.asarray(params["word_emb"], np.float32)
    pe = np.asarray(params["pos_emb"], np.float32)
    te = np.asarray(params["type_emb"], np.float32)
    x0 = we[ids] + pe[None, :S] + te[tt]  # [B, S, H] fp32
    x0T = np.ascontiguousarray(x0.transpose(0, 2, 1)).astype(ml_dtypes.bfloat16)

    in_maps = []
    for c in range(B):
        m = {"x0T": x0T[c]}
        for name in runner.in_names:
            if name != "x0T":
                m[name] = blobs[name]
        in_maps.append(m)
    res = runner.run(in_maps)
    logits = np.stack([res[c]["logits"] for c in range(B)]).astype(np.float32)
    logits = logits + np.asarray(params["cls_b"], np.float32)[None, None, :]
    return np.asarray(_crf_nll(params, logits, lab, mask), np.float32)
